# revision 1
# baseline (speedup 1.0000x reference)
"""CapsuleLayer (dynamic routing, 3 iterations) on 8 Trainium2 NeuronCores.

Decomposition (never materializes u_hat = [256,1152,10,16], 189MB):
  - Shard the 1152 input capsules (i) 8 ways: 144 per core.
  - Per-core row space j = (i_local, k), k = in_size = 8 -> 1152 rows
    = 9 chunks of 128 partitions.
  - s_j:  s[b,(n,o)] = sum_j xT[j,b] * (c[j,n] * Wl[j,(n,o)])   (PE matmul,
    contraction over j; Wl = 0.03*W in [(i,k),(n,o)] layout, c broadcast
    over k and o).  Partial over the i-shard -> exchanged across cores.
  - b_ij update via a Gram matrix instead of u_hat:
       Q[j,(n,o)]  = sum_b x[b,j] * v[b,(n,o)]                  (PE matmul)
       pr[j,n]     = sum_o Wl[j,(n,o)] * Q[j,(n,o)]             (DVE)
       uv_rows     = F.T @ pr  per 128-chunk, F = kron(I16, ones8x8)/B
                     (sums over k within each i-group AND replicates the
                     result back to all k-rows, so b stays row-replicated)
  - Iteration 1 uses uniform c = 1/10 (softmax of zeros): s1 = 0.1*(xT.T@Wl).
  - Iterations 1-2 exchange s partials with an fp8-e4m3 AllGather (cheaper
    than AllReduce on this stack) + on-chip tree-reduce; the rounding only
    perturbs the routing weights c_ij (~1e-4 on the final output).
  - Iteration 3 needs no b-update; the final fp32 s3 goes through
    ReduceScatter so each core squashes only its 32-row batch shard; the
    host just concatenates the 8 shards.

Precision plan: routing matmuls use bf16 operands (PSUM accumulates fp32;
fp32 matmuls on trn2 lower to 2x LDWEIGHTS + 2x dual-pass MATMUL, ~8x
slower).  The output-determining iteration-3 matmul uses a 3-product
Dekker split (xtH/xtL, mcH/mcL from an fp32 c3*Wl) so the bf16 PE
reproduces the fp32 result to ~1e-5.  sqrt is a bit-trick + Newton on the
DVE so the ScalarE only ever needs one activation-table set (Exp);
Sqrt/Ln live in other sets and would force ~2.7us ACT_TABLE_LOADs per
iteration.  A tiny warm-up AllGather at kernel start absorbs the one-time
ncfw/collective boot behind the input DMAs and first matmul phase.
"""
import sys

if "/opt/trn_rl_repo" not in sys.path:
    sys.path.insert(0, "/opt/trn_rl_repo")

import numpy as np

import os
N_CORES = int(os.environ.get("KERNEL_CORES", "8"))
B, IN_SIZE, I_TOT = 256, 8, 1152
N_NODE, O_SZ = 10, 16
NO = N_NODE * O_SZ          # 160
I_SH = I_TOT // N_CORES     # 144 capsules per core
JR = I_SH * IN_SIZE         # 1152 rows per core
NCH = JR // 128             # 9 contraction chunks
BC = B // 128               # 2 batch chunks
B_SH = B // N_CORES         # 32 batch rows per core after ReduceScatter

RSQRT_MAGIC = 0x5F3759DF
FAST_S3 = bool(int(os.environ.get("KERNEL_FAST_S3", "0")))

_CACHE = {}


def _build_program():
    import concourse.bacc as bacc
    import concourse.tile as tile
    import concourse.mybir as mybir

    f32 = mybir.dt.float32
    bf16 = mybir.dt.bfloat16
    f8 = mybir.dt.float8e4
    i32 = mybir.dt.int32
    AF = mybir.ActivationFunctionType
    ALU = mybir.AluOpType
    AX = mybir.AxisListType

    nc = bacc.Bacc("TRN2", target_bir_lowering=False, debug=False,
                   enable_asserts=True, num_devices=N_CORES)

    xt_d = nc.dram_tensor("xt", [JR, B], bf16, kind="ExternalInput").ap()
    xik_d = nc.dram_tensor("xik", [B, JR], bf16, kind="ExternalInput").ap()
    wl_d = nc.dram_tensor("wl", [JR, NO], bf16, kind="ExternalInput").ap()
    xtl_d = wlf_d = None
    if not FAST_S3:
        xtl_d = nc.dram_tensor("xtl", [JR, B], bf16,
                               kind="ExternalInput").ap()
        wlf_d = nc.dram_tensor("wlf", [JR, NO], f32,
                               kind="ExternalInput").ap()
    f_d = nc.dram_tensor("fmat", [128, 128], bf16, kind="ExternalInput").ap()
    y_d = nc.dram_tensor("y", [B_SH, NO], f32, kind="ExternalOutput").ap()

    RG = [list(range(N_CORES))]

    with tile.TileContext(nc) as tc:
        with tc.tile_pool(name="persist", bufs=1) as pp, \
             tc.tile_pool(name="work", bufs=1) as wp, \
             tc.tile_pool(name="ps_s", bufs=2, space="PSUM") as ps_s, \
             tc.tile_pool(name="ps_q", bufs=3, space="PSUM") as ps_q, \
             tc.tile_pool(name="ps_f", bufs=1, space="PSUM") as ps_f, \
             tc.tile_pool(name="dram", bufs=1, space="DRAM") as dp:

            # ---------------- input loads ----------------
            xt_sb = pp.tile([128, NCH, B], bf16, name="xt_sb", tag="xt_sb")
            if not FAST_S3:
                xtl_sb = pp.tile([128, NCH, B], bf16, name="xtl_sb",
                                 tag="xtl_sb")
            xik_sb = pp.tile([128, BC, JR], bf16, name="xik_sb", tag="xik_sb")
            wl_sb = pp.tile([128, NCH, NO], bf16, name="wl_sb", tag="wl_sb")
            if not FAST_S3:
                wlf_sb = pp.tile([128, NCH, NO], f32, name="wlf_sb",
                                 tag="wlf_sb")
            f_sb = pp.tile([128, 128], bf16, name="f_sb", tag="f_sb")
            b_sb = pp.tile([128, NCH, N_NODE], f32, name="b_sb", tag="b_sb")

            # Warm-up collective: absorbs the one-time ncfw/TOPSP collective
            # setup (and any cross-core launch skew) concurrently with the
            # input DMAs and the first matmul phase, so the first real
            # AllReduce doesn't pay it on the critical path.
            if int(os.environ.get("KERNEL_WARMUP", "1")):
                warm_in = dp.tile([128, 4], bf16, name="warm_in",
                                  tag="warm_in")
                warm_out = dp.tile([N_CORES * 128, 4], bf16, name="warm_out",
                                   tag="warm_out")
                nc.gpsimd.collective_compute(
                    "AllGather", ALU.bypass, replica_groups=RG,
                    ins=[warm_in.opt()], outs=[warm_out.opt()])

            # Spread input loads across engine DGE queues -- a single issuer
            # serializes ~600ns of descriptor work per DMA.
            engs = [nc.sync, nc.scalar, nc.gpsimd]
            # s1 needs xt+wl first; xik next (Q1); xtl/wlf/F much later.
            xt3 = xt_d.rearrange("(c p) b -> p c b", p=128)
            wl3 = wl_d.rearrange("(c p) f -> p c f", p=128)
            if not FAST_S3:
                xtl3 = xtl_d.rearrange("(c p) b -> p c b", p=128)
                wlf3 = wlf_d.rearrange("(c p) f -> p c f", p=128)
            for g, eng in [((0, 3), nc.sync), ((3, 6), nc.scalar),
                           ((6, NCH), nc.sync)]:
                eng.dma_start(xt_sb[:, g[0]:g[1], :], xt3[:, g[0]:g[1], :])
            for g, eng in [((0, 3), nc.scalar), ((3, 6), nc.sync),
                           ((6, NCH), nc.scalar)]:
                eng.dma_start(wl_sb[:, g[0]:g[1], :], wl3[:, g[0]:g[1], :])
            for bc_i in range(BC):
                engs[bc_i % 2].dma_start(xik_sb[:, bc_i, :],
                                         xik_d[bc_i * 128:(bc_i + 1) * 128, :])
            if not FAST_S3:
                nc.sync.dma_start(xtl_sb[:, 0:5, :], xtl3[:, 0:5, :])
                nc.scalar.dma_start(xtl_sb[:, 5:NCH, :], xtl3[:, 5:NCH, :])
                nc.sync.dma_start(wlf_sb[:, 0:5, :], wlf3[:, 0:5, :])
                nc.scalar.dma_start(wlf_sb[:, 5:NCH, :], wlf3[:, 5:NCH, :])
            nc.sync.dma_start(f_sb[:], f_d[:])

            wl4 = wl_sb[:].rearrange("p c (n o) -> p c n o", n=N_NODE)

            # ---------------- helpers ----------------
            def s_matmul(rhs3, s_sb, scale):
                """s_sb[:,bc,:] = scale * sum_c xt[:,c,bc].T @ rhs3[:,c,:]"""
                for bc_i in range(BC):
                    s_ps = ps_s.tile([128, NO], f32, name="s_ps", tag="s_ps")
                    for c in range(NCH):
                        nc.tensor.matmul(
                            s_ps[:],
                            xt_sb[:, c, bc_i * 128:(bc_i + 1) * 128],
                            rhs3[:, c, :],
                            start=(c == 0), stop=(c == NCH - 1))
                    if scale is None:
                        nc.scalar.copy(s_sb[:, bc_i, :], s_ps[:])
                    else:
                        nc.scalar.mul(s_sb[:, bc_i, :], s_ps[:], scale)

            def allgather_s(s_sb, t):
                """AllGather the bf16 s partials (AG is cheaper than
                AllReduce) and tree-reduce the 8 rank partials on the DVE.
                Payload stays in partition-major [128, BC*NO] layout so every
                DMA is a contiguous 2-D copy. Rounding here only perturbs the
                routing weights c_ij."""
                ag_in = dp.tile([128, BC * NO], f8, name=f"ag_in{t}",
                                tag="ag_in")
                ag_out = dp.tile([N_CORES * 128, BC * NO], f8,
                                 name=f"ag_out{t}", tag="ag_out")
                for bc_i in range(BC):
                    engs[bc_i % 2].dma_start(
                        ag_in[:, bc_i * NO:(bc_i + 1) * NO],
                        s_sb[:, bc_i, :])
                nc.gpsimd.collective_compute(
                    "AllGather", ALU.bypass, replica_groups=RG,
                    ins=[ag_in.opt()], outs=[ag_out.opt()])
                agv = wp.tile([128, N_CORES, BC * NO], f8, name="agv",
                              tag="agv")
                ag3 = ag_out.rearrange("(r p) f -> p r f", p=128)
                nh = N_CORES // 2
                for h in range(nh):
                    engs[h % 3].dma_start(agv[:, 2 * h:2 * h + 2, :],
                                          ag3[:, 2 * h:2 * h + 2, :])
                # leaf adds pair the two ranks of each DMA so the tree starts
                # as soon as individual transfers land
                t4 = wp.tile([128, nh, BC * NO], bf16, name="agt4", tag="agt4")
                for h in range(nh):
                    nc.vector.tensor_add(t4[:, h, :], agv[:, 2 * h, :],
                                         agv[:, 2 * h + 1, :])
                cur = t4[:]
                w = nh
                while w > 2:
                    w //= 2
                    nxt = wp.tile([128, w, BC * NO], bf16,
                                  name=f"agt{w}", tag=f"agt{w}")
                    nc.vector.tensor_add(nxt[:], cur[:, 0:w, :],
                                         cur[:, w:2 * w, :])
                    cur = nxt[:]
                sfull = wp.tile([128, BC, NO], bf16, name="sfull",
                                tag="sfull")
                nc.vector.tensor_add(
                    sfull[:].rearrange("p c f -> p (c f)"),
                    cur[:, 0, :], cur[:, 1, :])
                return sfull

            def rsqrt(msq, P, nch, tag, iters):
                """z ~ 1/sqrt(msq) via int bit-trick + Newton steps (DVE
                only -- avoids the Sqrt/Ln ACT table sets entirely)."""
                sh = [P, nch, N_NODE]
                zi = wp.tile(sh, i32, name="zi" + tag, tag="zi" + tag)
                # zi = ((bits >> 1) ^ -1) + (MAGIC + 1)  ==  MAGIC - (bits>>1)
                nc.vector.tensor_scalar(
                    out=zi[:], in0=msq[:].bitcast(i32), scalar1=1, scalar2=-1,
                    op0=ALU.arith_shift_right, op1=ALU.bitwise_xor)
                nc.vector.tensor_scalar_add(zi[:], zi[:], RSQRT_MAGIC + 1)
                z = zi[:].bitcast(f32)
                t = wp.tile(sh, f32, name="nt" + tag, tag="nt" + tag)
                w = wp.tile(sh, f32, name="nw" + tag, tag="nw" + tag)
                for _ in range(iters):
                    nc.vector.tensor_mul(t[:], z, z)
                    nc.vector.tensor_mul(t[:], t[:], msq[:])
                    nc.vector.tensor_scalar(
                        out=w[:], in0=t[:], scalar1=-0.5, scalar2=1.5,
                        op0=ALU.mult, op1=ALU.add)
                    nc.vector.tensor_mul(z, z, w[:])
                return z

            def squash(s_sb, P, nch, tag, v_dtype, newton_iters=1):
                """v = squash(s) over o. s_sb [P, nch, NO] fp32. One Newton
                step (~0.2% rsqrt error) suffices for the routing iterations;
                the output pass uses two (~1e-5)."""
                s4 = s_sb[:].rearrange("p c (n o) -> p c n o", n=N_NODE)
                sq = wp.tile([P, nch, NO], s_sb.dtype, name="sq" + tag,
                             tag="sq" + tag)
                nc.vector.tensor_mul(sq[:], s_sb[:], s_sb[:])
                msq = wp.tile([P, nch, N_NODE], f32, name="msq" + tag,
                              tag="msq" + tag)
                nc.vector.reduce_sum(
                    msq[:], sq[:].rearrange("p c (n o) -> p c n o", n=N_NODE),
                    axis=AX.X)
                z = rsqrt(msq, P, nch, tag, newton_iters)
                mag = wp.tile([P, nch, N_NODE], f32, name="mag" + tag,
                              tag="mag" + tag)
                nc.vector.tensor_mul(mag[:], msq[:], z)   # sqrt(msq)
                den = wp.tile([P, nch, N_NODE], f32, name="den" + tag,
                              tag="den" + tag)
                nc.vector.tensor_scalar_add(den[:], msq[:], 1.0)
                rden = wp.tile([P, nch, N_NODE], f32, name="rden" + tag,
                               tag="rden" + tag)
                nc.vector.reciprocal(rden[:], den[:])
                fac = wp.tile([P, nch, N_NODE], f32, name="fac" + tag,
                              tag="fac" + tag)
                nc.vector.tensor_mul(fac[:], mag[:], rden[:])
                v_sb = wp.tile([P, nch, NO], v_dtype, name="v" + tag,
                               tag="v" + tag)
                fb = fac[:].unsqueeze(3).broadcast_to((P, nch, N_NODE, O_SZ))
                nc.vector.tensor_mul(
                    v_sb[:].rearrange("p c (n o) -> p c n o", n=N_NODE), s4, fb)
                return v_sb

            def b_update(v_sb, first):
                # Q matmuls pack 3 j-chunks per PSUM bank; p = wl * Q reads
                # each bank straight out of PSUM (3 wide TTs, no Q copies).
                p_sb = wp.tile([128, NCH, NO], bf16, name="p_sb", tag="p_sb")
                for g in range(NCH // 3):
                    q_ps = ps_q.tile([128, 3 * NO], f32, name="q_ps",
                                     tag="q_ps")
                    for s_i in range(3):
                        mc = g * 3 + s_i
                        for bc_i in range(BC):
                            nc.tensor.matmul(
                                q_ps[:, s_i * NO:(s_i + 1) * NO],
                                xik_sb[:, bc_i, mc * 128:(mc + 1) * 128],
                                v_sb[:, bc_i, :],
                                start=(bc_i == 0), stop=(bc_i == BC - 1))
                    nc.vector.tensor_mul(
                        p_sb[:, g * 3:(g + 1) * 3, :],
                        wl_sb[:, g * 3:(g + 1) * 3, :],
                        q_ps[:].rearrange("p (c f) -> p c f", c=3))
                pr = wp.tile([128, NCH, N_NODE], f32, name="pr_sb", tag="pr_sb")
                for g in range(NCH // 3):
                    nc.vector.reduce_sum(
                        pr[:, g * 3:(g + 1) * 3, :],
                        p_sb[:, g * 3:(g + 1) * 3, :].rearrange(
                            "p c (n o) -> p c n o", n=N_NODE),
                        axis=AX.X)
                prb = wp.tile([128, NCH, N_NODE], bf16, name="prb", tag="prb")
                nc.vector.tensor_copy(prb[:], pr[:])
                uv_ps = ps_f.tile([128, NCH * N_NODE], f32, name="uv_ps",
                                  tag="uv_ps")
                nc.tensor.matmul(uv_ps[:], f_sb[:],
                                 prb[:].rearrange("p c n -> p (c n)"),
                                 start=True, stop=True)
                uv3 = uv_ps[:].rearrange("p (c n) -> p c n", n=N_NODE)
                if first:
                    # keep b state for the next update, but let the softmax
                    # read the PSUM uv directly (shorter critical path)
                    nc.scalar.copy(b_sb[:], uv3)
                    return uv3
                nc.vector.tensor_add(b_sb[:], b_sb[:], uv3)
                return b_sb[:]

            def softmax_c(c_dtype, b_src):
                e_sb = wp.tile([128, NCH, N_NODE], f32, name="e_sb", tag="e_sb")
                nc.scalar.activation(e_sb[:], b_src, AF.Exp)
                se = wp.tile([128, NCH], f32, name="se", tag="se")
                nc.vector.reduce_sum(se[:], e_sb[:], axis=AX.X)
                rse = wp.tile([128, NCH], f32, name="rse", tag="rse")
                nc.vector.reciprocal(rse[:], se[:])
                c_sb = wp.tile([128, NCH, N_NODE], c_dtype, name="c_sb",
                               tag="c_sb" + str(c_dtype))
                nc.vector.tensor_mul(
                    c_sb[:], e_sb[:],
                    rse[:].unsqueeze(2).broadcast_to((128, NCH, N_NODE)))
                return c_sb

            def softmax_mc(b_src):
                c_sb = softmax_c(bf16, b_src)
                mc_sb = wp.tile([128, NCH, NO], bf16, name="mc_sb", tag="mc_sb")
                cb = c_sb[:].unsqueeze(3).broadcast_to(
                    (128, NCH, N_NODE, O_SZ))
                mc4 = mc_sb[:].rearrange("p c (n o) -> p c n o", n=N_NODE)
                # split the W-sized multiply across DVE and the idle GpSimd
                nc.vector.tensor_mul(mc4[:, 0:6], wl4[:, 0:6], cb[:, 0:6])
                nc.gpsimd.tensor_mul(mc4[:, 6:NCH], wl4[:, 6:NCH],
                                     cb[:, 6:NCH])
                return mc_sb

            def _dekker_s3(b_src):
                # fp32 c3/mc3, then a 3-product Dekker split so the bf16 PE
                # reproduces the fp32 matmul to ~1e-5:
                #   s3 = xtH.T @ mcH  +  xtH.T @ mcL  +  xtL.T @ mcH
                c3 = softmax_c(f32, b_src)
                mc3 = wp.tile([128, NCH, NO], f32, name="mc3", tag="mc3")
                cb3 = c3[:].unsqueeze(3).broadcast_to(
                    (128, NCH, N_NODE, O_SZ))
                wlf4 = wlf_sb[:].rearrange("p c (n o) -> p c n o", n=N_NODE)
                mc34 = mc3[:].rearrange("p c (n o) -> p c n o", n=N_NODE)
                # hi/lo split, chunk-group-pipelined so the PE can start on
                # early chunks while later ones are still being built
                mcp = wp.tile([128, NCH, 2, NO], bf16, name="mcp", tag="mcp")
                for g in range(NCH // 3):
                    gs = slice(g * 3, (g + 1) * 3)
                    nc.vector.tensor_mul(mc34[:, gs], wlf4[:, gs], cb3[:, gs])
                    nc.scalar.copy(mcp[:, gs, 0, :], mc3[:, gs, :])
                    nc.gpsimd.tensor_sub(mcp[:, gs, 1, :], mc3[:, gs, :],
                                         mcp[:, gs, 0, :])
                s_sb = wp.tile([128, BC, NO], f32, name="s_sb", tag="s_sb")
                for bc_i in range(BC):
                    ps_a = ps_s.tile([128, 2 * NO], f32, name="ps_a",
                                     tag="ps_a")
                    ps_c = ps_s.tile([128, NO], f32, name="s_ps", tag="s_ps")
                    for c in range(NCH):
                        lhs_h = xt_sb[:, c, bc_i * 128:(bc_i + 1) * 128]
                        lhs_l = xtl_sb[:, c, bc_i * 128:(bc_i + 1) * 128]
                        nc.tensor.matmul(
                            ps_a[:], lhs_h,
                            mcp[:, c, :, :].rearrange("p t f -> p (t f)"),
                            start=(c == 0), stop=(c == NCH - 1))
                        nc.tensor.matmul(
                            ps_c[:], lhs_l, mcp[:, c, 0, :],
                            start=(c == 0), stop=(c == NCH - 1))
                    nc.scalar.copy(s_sb[:, bc_i, :], ps_a[:, 0:NO])
                    nc.vector.tensor_add(s_sb[:, bc_i, :], s_sb[:, bc_i, :],
                                         ps_a[:, NO:2 * NO])
                    nc.vector.tensor_add(s_sb[:, bc_i, :], s_sb[:, bc_i, :],
                                         ps_c[:])
                return s_sb

            # ---------------- iteration 1 (c uniform = 0.1) ----------------
            s_sb = wp.tile([128, BC, NO], f8, name="s_sbr", tag="s_sbr")
            s_matmul(wl_sb[:], s_sb, scale=0.1)
            sfull = allgather_s(s_sb, 0)
            v_sb = squash(sfull, 128, BC, "m", bf16)
            b_src = b_update(v_sb, first=True)

            # ---------------- iteration 2 ----------------
            mc_sb = softmax_mc(b_src)
            s_sb = wp.tile([128, BC, NO], f8, name="s_sbr", tag="s_sbr")
            s_matmul(mc_sb[:], s_sb, scale=None)
            sfull = allgather_s(s_sb, 1)
            v_sb = squash(sfull, 128, BC, "m", bf16)
            b_src = b_update(v_sb, first=False)

            # ---------------- iteration 3 (no b-update) ----------------
            if FAST_S3:
                mc_sb = softmax_mc(b_src)
                s_sb = wp.tile([128, BC, NO], f32, name="s_sb", tag="s_sb")
                s_matmul(mc_sb[:], s_sb, scale=None)
            else:
                s_sb = _dekker_s3(b_src)

            rs_in = dp.tile([B, NO], f32, name="rs_in", tag="rs_in")
            rs_out = dp.tile([B_SH, NO], f32, name="rs_out", tag="rs_out")
            for bc_i in range(BC):
                engs[bc_i % 2].dma_start(
                    rs_in[bc_i * 128:(bc_i + 1) * 128, :], s_sb[:, bc_i, :])
            nc.gpsimd.collective_compute(
                "ReduceScatter", ALU.add, replica_groups=RG,
                ins=[rs_in.opt()], outs=[rs_out.opt()])
            ssh = wp.tile([B_SH, 1, NO], f32, name="ssh", tag="ssh")
            nc.sync.dma_start(ssh[:, 0, :], rs_out[:])
            vsh = squash(ssh, B_SH, 1, "s", f32, newton_iters=2)
            nc.sync.dma_start(y_d[:], vsh[:, 0, :])
            rs_in = dp.tile([B, NO], f32, name="rs_in", tag="rs_in")
            rs_out = dp.tile([B_SH, NO], f32, name="rs_out", tag="rs_out")
            for bc_i in range(BC):
                engs[bc_i % 2].dma_start(
                    rs_in[bc_i * 128:(bc_i + 1) * 128, :], s_sb[:, bc_i, :])
            nc.gpsimd.collective_compute(
                "ReduceScatter", ALU.add, replica_groups=RG,
                ins=[rs_in.opt()], outs=[rs_out.opt()])
            ssh = wp.tile([B_SH, 1, NO], f32, name="ssh", tag="ssh")
            nc.sync.dma_start(ssh[:, 0, :], rs_out[:])
            vsh = squash(ssh, B_SH, 1, "s", f32, newton_iters=2)
            nc.sync.dma_start(y_d[:], vsh[:, 0, :])

    nc.compile()
    return nc


def _host_prep(x, W):
    """Per-core input dicts + the constant F matrix."""
    import ml_dtypes

    bf = ml_dtypes.bfloat16
    x = np.ascontiguousarray(x, dtype=np.float32)
    W = np.ascontiguousarray(W, dtype=np.float32)
    F = (np.kron(np.eye(16, dtype=np.float32),
                 np.ones((8, 8), dtype=np.float32)) / np.float32(B)).astype(bf)
    in_maps = []
    for c in range(N_CORES):
        sl = slice(c * I_SH, (c + 1) * I_SH)
        x_sh = x[:, :, sl]                                   # [B, K, I_SH]
        xt = np.ascontiguousarray(x_sh.transpose(2, 1, 0)).reshape(JR, B)
        xt_hi = xt.astype(bf)
        xt_lo = (xt - xt_hi.astype(np.float32)).astype(bf)
        xik = np.ascontiguousarray(
            x_sh.transpose(0, 2, 1)).reshape(B, JR).astype(bf)
        wlf = np.ascontiguousarray(
            (np.float32(0.03) * W[0, sl]).transpose(0, 3, 1, 2)
        ).reshape(JR, NO)
        m = {"xt": xt_hi, "xik": xik, "wl": wlf.astype(bf), "fmat": F}
        if not FAST_S3:
            m["xtl"] = xt_lo
            m["wlf"] = wlf
        in_maps.append(m)
    return in_maps


def _run(in_maps, trace=False, all_cores=False):
    from concourse.bass_utils import run_bass_kernel_spmd

    if "nc" not in _CACHE:
        _CACHE["nc"] = _build_program()
    nc = _CACHE["nc"]
    kwargs = {}
    if all_cores:
        kwargs["trace_cores"] = list(range(N_CORES))
    res = run_bass_kernel_spmd(nc, in_maps, core_ids=list(range(N_CORES)),
                               trace=trace, **kwargs)
    return res


def kernel(x: np.ndarray, W: np.ndarray) -> np.ndarray:
    in_maps = _host_prep(x, W)
    res = _run(in_maps)
    v = np.concatenate([res.results[c]["y"] for c in range(N_CORES)], axis=0)
    return v.reshape(B, N_NODE, O_SZ, 1).astype(np.float32)



# revision 2
# speedup vs baseline: 1.0026x; 1.0026x over previous
"""CapsuleLayer (dynamic routing, 3 iterations) on 8 Trainium2 NeuronCores.

Decomposition (never materializes u_hat = [256,1152,10,16], 189MB):
  - Shard the 1152 input capsules (i) 8 ways: 144 per core.
  - Per-core row space j = (i_local, k), k = in_size = 8 -> 1152 rows
    = 9 chunks of 128 partitions.
  - s_j:  s[b,(n,o)] = sum_j xT[j,b] * (c[j,n] * Wl[j,(n,o)])   (PE matmul,
    contraction over j; Wl = 0.03*W in [(i,k),(n,o)] layout, c broadcast
    over k and o).  Partial over the i-shard -> exchanged across cores.
  - b_ij update via a Gram matrix instead of u_hat:
       Q[j,(n,o)]  = sum_b x[b,j] * v[b,(n,o)]                  (PE matmul)
       pr[j,n]     = sum_o Wl[j,(n,o)] * Q[j,(n,o)]             (DVE)
       uv_rows     = F.T @ pr  per 128-chunk, F = kron(I16, ones8x8)/B
                     (sums over k within each i-group AND replicates the
                     result back to all k-rows, so b stays row-replicated)
  - Iteration 1 uses uniform c = 1/10 (softmax of zeros): s1 = 0.1*(xT.T@Wl).
  - Iterations 1-2 exchange s partials with an fp8-e4m3 AllGather (cheaper
    than AllReduce on this stack) + on-chip tree-reduce; the rounding only
    perturbs the routing weights c_ij (~1e-4 on the final output).
  - Iteration 3 needs no b-update; the final fp32 s3 goes through
    ReduceScatter so each core squashes only its 32-row batch shard; the
    host just concatenates the 8 shards.

Precision plan: routing matmuls use bf16 operands (PSUM accumulates fp32;
fp32 matmuls on trn2 lower to 2x LDWEIGHTS + 2x dual-pass MATMUL, ~8x
slower).  The output-determining iteration-3 matmul uses a 3-product
Dekker split (xtH/xtL, mcH/mcL from an fp32 c3*Wl) so the bf16 PE
reproduces the fp32 result to ~1e-5.  sqrt is a bit-trick + Newton on the
DVE so the ScalarE only ever needs one activation-table set (Exp);
Sqrt/Ln live in other sets and would force ~2.7us ACT_TABLE_LOADs per
iteration.  A tiny warm-up AllGather at kernel start absorbs the one-time
ncfw/collective boot behind the input DMAs and first matmul phase.
"""
import sys

if "/opt/trn_rl_repo" not in sys.path:
    sys.path.insert(0, "/opt/trn_rl_repo")

import numpy as np

import os
N_CORES = int(os.environ.get("KERNEL_CORES", "8"))
B, IN_SIZE, I_TOT = 256, 8, 1152
N_NODE, O_SZ = 10, 16
NO = N_NODE * O_SZ          # 160
I_SH = I_TOT // N_CORES     # 144 capsules per core
JR = I_SH * IN_SIZE         # 1152 rows per core
NCH = JR // 128             # 9 contraction chunks
BC = B // 128               # 2 batch chunks
B_SH = B // N_CORES         # 32 batch rows per core after ReduceScatter

RSQRT_MAGIC = 0x5F3759DF
FAST_S3 = bool(int(os.environ.get("KERNEL_FAST_S3", "0")))

_CACHE = {}


def _build_program():
    import concourse.bacc as bacc
    import concourse.tile as tile
    import concourse.mybir as mybir

    f32 = mybir.dt.float32
    bf16 = mybir.dt.bfloat16
    f8 = mybir.dt.float8e4
    i32 = mybir.dt.int32
    AF = mybir.ActivationFunctionType
    ALU = mybir.AluOpType
    AX = mybir.AxisListType

    nc = bacc.Bacc("TRN2", target_bir_lowering=False, debug=False,
                   enable_asserts=True, num_devices=N_CORES)

    xt_d = nc.dram_tensor("xt", [JR, B], bf16, kind="ExternalInput").ap()
    xik_d = nc.dram_tensor("xik", [B, JR], bf16, kind="ExternalInput").ap()
    wl_d = nc.dram_tensor("wl", [JR, NO], bf16, kind="ExternalInput").ap()
    xtl_d = wlf_d = None
    if not FAST_S3:
        xtl_d = nc.dram_tensor("xtl", [JR, B], bf16,
                               kind="ExternalInput").ap()
        wlf_d = nc.dram_tensor("wlf", [JR, NO], f32,
                               kind="ExternalInput").ap()
    f_d = nc.dram_tensor("fmat", [128, 128], bf16, kind="ExternalInput").ap()
    y_d = nc.dram_tensor("y", [B_SH, NO], f32, kind="ExternalOutput").ap()

    RG = [list(range(N_CORES))]

    with tile.TileContext(nc) as tc:
        with tc.tile_pool(name="persist", bufs=1) as pp, \
             tc.tile_pool(name="work", bufs=1) as wp, \
             tc.tile_pool(name="ps_s", bufs=2, space="PSUM") as ps_s, \
             tc.tile_pool(name="ps_q", bufs=3, space="PSUM") as ps_q, \
             tc.tile_pool(name="ps_f", bufs=1, space="PSUM") as ps_f, \
             tc.tile_pool(name="dram", bufs=1, space="DRAM") as dp:

            # ---------------- input loads ----------------
            xt_sb = pp.tile([128, NCH, B], bf16, name="xt_sb", tag="xt_sb")
            if not FAST_S3:
                xtl_sb = pp.tile([128, NCH, B], bf16, name="xtl_sb",
                                 tag="xtl_sb")
            xik_sb = pp.tile([128, BC, JR], bf16, name="xik_sb", tag="xik_sb")
            wl_sb = pp.tile([128, NCH, NO], bf16, name="wl_sb", tag="wl_sb")
            if not FAST_S3:
                wlf_sb = pp.tile([128, NCH, NO], f32, name="wlf_sb",
                                 tag="wlf_sb")
            f_sb = pp.tile([128, 128], bf16, name="f_sb", tag="f_sb")
            b_sb = pp.tile([128, NCH, N_NODE], f32, name="b_sb", tag="b_sb")

            # Warm-up collective: absorbs the one-time ncfw/TOPSP collective
            # setup (and any cross-core launch skew) concurrently with the
            # input DMAs and the first matmul phase, so the first real
            # AllReduce doesn't pay it on the critical path.
            if int(os.environ.get("KERNEL_WARMUP", "1")):
                warm_in = dp.tile([128, 4], bf16, name="warm_in",
                                  tag="warm_in")
                warm_out = dp.tile([N_CORES * 128, 4], bf16, name="warm_out",
                                   tag="warm_out")
                nc.gpsimd.collective_compute(
                    "AllGather", ALU.bypass, replica_groups=RG,
                    ins=[warm_in.opt()], outs=[warm_out.opt()])

            # Spread input loads across engine DGE queues -- a single issuer
            # serializes ~600ns of descriptor work per DMA.
            engs = [nc.sync, nc.scalar, nc.gpsimd]
            # s1 needs xt+wl first; xik next (Q1); xtl/wlf/F much later.
            xt3 = xt_d.rearrange("(c p) b -> p c b", p=128)
            wl3 = wl_d.rearrange("(c p) f -> p c f", p=128)
            if not FAST_S3:
                xtl3 = xtl_d.rearrange("(c p) b -> p c b", p=128)
                wlf3 = wlf_d.rearrange("(c p) f -> p c f", p=128)
            for g, eng in [((0, 3), nc.sync), ((3, 6), nc.scalar),
                           ((6, NCH), nc.sync)]:
                eng.dma_start(xt_sb[:, g[0]:g[1], :], xt3[:, g[0]:g[1], :])
            for g, eng in [((0, 3), nc.scalar), ((3, 6), nc.sync),
                           ((6, NCH), nc.scalar)]:
                eng.dma_start(wl_sb[:, g[0]:g[1], :], wl3[:, g[0]:g[1], :])
            for bc_i in range(BC):
                engs[bc_i % 2].dma_start(xik_sb[:, bc_i, :],
                                         xik_d[bc_i * 128:(bc_i + 1) * 128, :])
            if not FAST_S3:
                nc.sync.dma_start(xtl_sb[:, 0:5, :], xtl3[:, 0:5, :])
                nc.scalar.dma_start(xtl_sb[:, 5:NCH, :], xtl3[:, 5:NCH, :])
                nc.sync.dma_start(wlf_sb[:, 0:5, :], wlf3[:, 0:5, :])
                nc.scalar.dma_start(wlf_sb[:, 5:NCH, :], wlf3[:, 5:NCH, :])
            nc.sync.dma_start(f_sb[:], f_d[:])

            wl4 = wl_sb[:].rearrange("p c (n o) -> p c n o", n=N_NODE)

            # ---------------- helpers ----------------
            def s_matmul(rhs3, s_sb, scale):
                """s_sb[:,bc,:] = scale * sum_c xt[:,c,bc].T @ rhs3[:,c,:]"""
                for bc_i in range(BC):
                    s_ps = ps_s.tile([128, NO], f32, name="s_ps", tag="s_ps")
                    for c in range(NCH):
                        nc.tensor.matmul(
                            s_ps[:],
                            xt_sb[:, c, bc_i * 128:(bc_i + 1) * 128],
                            rhs3[:, c, :],
                            start=(c == 0), stop=(c == NCH - 1))
                    if scale is None:
                        nc.scalar.copy(s_sb[:, bc_i, :], s_ps[:])
                    else:
                        nc.scalar.mul(s_sb[:, bc_i, :], s_ps[:], scale)

            def allgather_s(s_sb, t):
                """AllGather the bf16 s partials (AG is cheaper than
                AllReduce) and tree-reduce the 8 rank partials on the DVE.
                Payload stays in partition-major [128, BC*NO] layout so every
                DMA is a contiguous 2-D copy. Rounding here only perturbs the
                routing weights c_ij."""
                ag_in = dp.tile([128, BC * NO], f8, name=f"ag_in{t}",
                                tag="ag_in")
                ag_out = dp.tile([N_CORES * 128, BC * NO], f8,
                                 name=f"ag_out{t}", tag="ag_out")
                for bc_i in range(BC):
                    engs[bc_i % 2].dma_start(
                        ag_in[:, bc_i * NO:(bc_i + 1) * NO],
                        s_sb[:, bc_i, :])
                nc.gpsimd.collective_compute(
                    "AllGather", ALU.bypass, replica_groups=RG,
                    ins=[ag_in.opt()], outs=[ag_out.opt()])
                agv = wp.tile([128, N_CORES, BC * NO], f8, name="agv",
                              tag="agv")
                ag3 = ag_out.rearrange("(r p) f -> p r f", p=128)
                nh = N_CORES // 2
                for h in range(nh):
                    engs[h % 3].dma_start(agv[:, 2 * h:2 * h + 2, :],
                                          ag3[:, 2 * h:2 * h + 2, :])
                # leaf adds pair the two ranks of each DMA so the tree starts
                # as soon as individual transfers land
                t4 = wp.tile([128, nh, BC * NO], bf16, name="agt4", tag="agt4")
                for h in range(nh):
                    nc.vector.tensor_add(t4[:, h, :], agv[:, 2 * h, :],
                                         agv[:, 2 * h + 1, :])
                cur = t4[:]
                w = nh
                while w > 2:
                    w //= 2
                    nxt = wp.tile([128, w, BC * NO], bf16,
                                  name=f"agt{w}", tag=f"agt{w}")
                    nc.vector.tensor_add(nxt[:], cur[:, 0:w, :],
                                         cur[:, w:2 * w, :])
                    cur = nxt[:]
                sfull = wp.tile([128, BC, NO], bf16, name="sfull",
                                tag="sfull")
                nc.vector.tensor_add(
                    sfull[:].rearrange("p c f -> p (c f)"),
                    cur[:, 0, :], cur[:, 1, :])
                return sfull

            def rsqrt(msq, P, nch, tag, iters):
                """z ~ 1/sqrt(msq) via int bit-trick + Newton steps (DVE
                only -- avoids the Sqrt/Ln ACT table sets entirely)."""
                sh = [P, nch, N_NODE]
                zi = wp.tile(sh, i32, name="zi" + tag, tag="zi" + tag)
                # zi = ((bits >> 1) ^ -1) + (MAGIC + 1)  ==  MAGIC - (bits>>1)
                nc.vector.tensor_scalar(
                    out=zi[:], in0=msq[:].bitcast(i32), scalar1=1, scalar2=-1,
                    op0=ALU.arith_shift_right, op1=ALU.bitwise_xor)
                nc.vector.tensor_scalar_add(zi[:], zi[:], RSQRT_MAGIC + 1)
                z = zi[:].bitcast(f32)
                t = wp.tile(sh, f32, name="nt" + tag, tag="nt" + tag)
                w = wp.tile(sh, f32, name="nw" + tag, tag="nw" + tag)
                for _ in range(iters):
                    nc.vector.tensor_mul(t[:], z, z)
                    nc.vector.tensor_mul(t[:], t[:], msq[:])
                    nc.vector.tensor_scalar(
                        out=w[:], in0=t[:], scalar1=-0.5, scalar2=1.5,
                        op0=ALU.mult, op1=ALU.add)
                    nc.vector.tensor_mul(z, z, w[:])
                return z

            def squash(s_sb, P, nch, tag, v_dtype, newton_iters=1):
                """v = squash(s) over o. s_sb [P, nch, NO] fp32. One Newton
                step (~0.2% rsqrt error) suffices for the routing iterations;
                the output pass uses two (~1e-5)."""
                s4 = s_sb[:].rearrange("p c (n o) -> p c n o", n=N_NODE)
                sq = wp.tile([P, nch, NO], s_sb.dtype, name="sq" + tag,
                             tag="sq" + tag)
                nc.vector.tensor_mul(sq[:], s_sb[:], s_sb[:])
                msq = wp.tile([P, nch, N_NODE], f32, name="msq" + tag,
                              tag="msq" + tag)
                nc.vector.reduce_sum(
                    msq[:], sq[:].rearrange("p c (n o) -> p c n o", n=N_NODE),
                    axis=AX.X)
                z = rsqrt(msq, P, nch, tag, newton_iters)
                mag = wp.tile([P, nch, N_NODE], f32, name="mag" + tag,
                              tag="mag" + tag)
                nc.vector.tensor_mul(mag[:], msq[:], z)   # sqrt(msq)
                den = wp.tile([P, nch, N_NODE], f32, name="den" + tag,
                              tag="den" + tag)
                nc.vector.tensor_scalar_add(den[:], msq[:], 1.0)
                rden = wp.tile([P, nch, N_NODE], f32, name="rden" + tag,
                               tag="rden" + tag)
                nc.vector.reciprocal(rden[:], den[:])
                fac = wp.tile([P, nch, N_NODE], f32, name="fac" + tag,
                              tag="fac" + tag)
                nc.vector.tensor_mul(fac[:], mag[:], rden[:])
                v_sb = wp.tile([P, nch, NO], v_dtype, name="v" + tag,
                               tag="v" + tag)
                fb = fac[:].unsqueeze(3).broadcast_to((P, nch, N_NODE, O_SZ))
                nc.vector.tensor_mul(
                    v_sb[:].rearrange("p c (n o) -> p c n o", n=N_NODE), s4, fb)
                return v_sb

            def b_update(v_sb, first):
                # Q matmuls pack 3 j-chunks per PSUM bank; p = wl * Q reads
                # each bank straight out of PSUM (3 wide TTs, no Q copies).
                p_sb = wp.tile([128, NCH, NO], bf16, name="p_sb", tag="p_sb")
                for g in range(NCH // 3):
                    q_ps = ps_q.tile([128, 3 * NO], f32, name="q_ps",
                                     tag="q_ps")
                    for s_i in range(3):
                        mc = g * 3 + s_i
                        for bc_i in range(BC):
                            nc.tensor.matmul(
                                q_ps[:, s_i * NO:(s_i + 1) * NO],
                                xik_sb[:, bc_i, mc * 128:(mc + 1) * 128],
                                v_sb[:, bc_i, :],
                                start=(bc_i == 0), stop=(bc_i == BC - 1))
                    nc.vector.tensor_mul(
                        p_sb[:, g * 3:(g + 1) * 3, :],
                        wl_sb[:, g * 3:(g + 1) * 3, :],
                        q_ps[:].rearrange("p (c f) -> p c f", c=3))
                pr = wp.tile([128, NCH, N_NODE], f32, name="pr_sb", tag="pr_sb")
                for g in range(NCH // 3):
                    nc.vector.reduce_sum(
                        pr[:, g * 3:(g + 1) * 3, :],
                        p_sb[:, g * 3:(g + 1) * 3, :].rearrange(
                            "p c (n o) -> p c n o", n=N_NODE),
                        axis=AX.X)
                prb = wp.tile([128, NCH, N_NODE], bf16, name="prb", tag="prb")
                nc.vector.tensor_copy(prb[:], pr[:])
                uv_ps = ps_f.tile([128, NCH * N_NODE], f32, name="uv_ps",
                                  tag="uv_ps")
                nc.tensor.matmul(uv_ps[:], f_sb[:],
                                 prb[:].rearrange("p c n -> p (c n)"),
                                 start=True, stop=True)
                uv3 = uv_ps[:].rearrange("p (c n) -> p c n", n=N_NODE)
                if first:
                    # keep b state for the next update, but let the softmax
                    # read the PSUM uv directly (shorter critical path)
                    nc.scalar.copy(b_sb[:], uv3)
                    return uv3
                nc.vector.tensor_add(b_sb[:], b_sb[:], uv3)
                return b_sb[:]

            def softmax_c(c_dtype, b_src):
                e_sb = wp.tile([128, NCH, N_NODE], f32, name="e_sb", tag="e_sb")
                nc.scalar.activation(e_sb[:], b_src, AF.Exp)
                se = wp.tile([128, NCH], f32, name="se", tag="se")
                nc.vector.reduce_sum(se[:], e_sb[:], axis=AX.X)
                rse = wp.tile([128, NCH], f32, name="rse", tag="rse")
                nc.vector.reciprocal(rse[:], se[:])
                c_sb = wp.tile([128, NCH, N_NODE], c_dtype, name="c_sb",
                               tag="c_sb" + str(c_dtype))
                nc.vector.tensor_mul(
                    c_sb[:], e_sb[:],
                    rse[:].unsqueeze(2).broadcast_to((128, NCH, N_NODE)))
                return c_sb

            def softmax_mc(b_src):
                c_sb = softmax_c(bf16, b_src)
                mc_sb = wp.tile([128, NCH, NO], bf16, name="mc_sb", tag="mc_sb")
                cb = c_sb[:].unsqueeze(3).broadcast_to(
                    (128, NCH, N_NODE, O_SZ))
                mc4 = mc_sb[:].rearrange("p c (n o) -> p c n o", n=N_NODE)
                # split the W-sized multiply across DVE and the idle GpSimd
                nc.vector.tensor_mul(mc4[:, 0:6], wl4[:, 0:6], cb[:, 0:6])
                nc.gpsimd.tensor_mul(mc4[:, 6:NCH], wl4[:, 6:NCH],
                                     cb[:, 6:NCH])
                return mc_sb

            def _dekker_s3(b_src):
                # fp32 c3/mc3, then a 3-product Dekker split so the bf16 PE
                # reproduces the fp32 matmul to ~1e-5:
                #   s3 = xtH.T @ mcH  +  xtH.T @ mcL  +  xtL.T @ mcH
                c3 = softmax_c(f32, b_src)
                mc3 = wp.tile([128, NCH, NO], f32, name="mc3", tag="mc3")
                cb3 = c3[:].unsqueeze(3).broadcast_to(
                    (128, NCH, N_NODE, O_SZ))
                wlf4 = wlf_sb[:].rearrange("p c (n o) -> p c n o", n=N_NODE)
                mc34 = mc3[:].rearrange("p c (n o) -> p c n o", n=N_NODE)
                # hi/lo split, chunk-group-pipelined so the PE can start on
                # early chunks while later ones are still being built
                mcp = wp.tile([128, NCH, 2, NO], bf16, name="mcp", tag="mcp")
                for g in range(NCH // 3):
                    gs = slice(g * 3, (g + 1) * 3)
                    nc.vector.tensor_mul(mc34[:, gs], wlf4[:, gs], cb3[:, gs])
                    nc.scalar.copy(mcp[:, gs, 0, :], mc3[:, gs, :])
                    nc.gpsimd.tensor_sub(mcp[:, gs, 1, :], mc3[:, gs, :],
                                         mcp[:, gs, 0, :])
                s_sb = wp.tile([128, BC, NO], f32, name="s_sb", tag="s_sb")
                for bc_i in range(BC):
                    ps_a = ps_s.tile([128, 2 * NO], f32, name="ps_a",
                                     tag="ps_a")
                    ps_c = ps_s.tile([128, NO], f32, name="s_ps", tag="s_ps")
                    for c in range(NCH):
                        lhs_h = xt_sb[:, c, bc_i * 128:(bc_i + 1) * 128]
                        lhs_l = xtl_sb[:, c, bc_i * 128:(bc_i + 1) * 128]
                        nc.tensor.matmul(
                            ps_a[:], lhs_h,
                            mcp[:, c, :, :].rearrange("p t f -> p (t f)"),
                            start=(c == 0), stop=(c == NCH - 1))
                        nc.tensor.matmul(
                            ps_c[:], lhs_l, mcp[:, c, 0, :],
                            start=(c == 0), stop=(c == NCH - 1))
                    nc.scalar.copy(s_sb[:, bc_i, :], ps_a[:, 0:NO])
                    nc.vector.tensor_add(s_sb[:, bc_i, :], s_sb[:, bc_i, :],
                                         ps_a[:, NO:2 * NO])
                    nc.vector.tensor_add(s_sb[:, bc_i, :], s_sb[:, bc_i, :],
                                         ps_c[:])
                return s_sb

            # ---------------- iteration 1 (c uniform = 0.1) ----------------
            s_sb = wp.tile([128, BC, NO], f8, name="s_sbr", tag="s_sbr")
            s_matmul(wl_sb[:], s_sb, scale=0.1)
            sfull = allgather_s(s_sb, 0)
            v_sb = squash(sfull, 128, BC, "m", bf16)
            b_src = b_update(v_sb, first=True)

            # ---------------- iteration 2 ----------------
            mc_sb = softmax_mc(b_src)
            s_sb = wp.tile([128, BC, NO], f8, name="s_sbr", tag="s_sbr")
            s_matmul(mc_sb[:], s_sb, scale=None)
            sfull = allgather_s(s_sb, 1)
            v_sb = squash(sfull, 128, BC, "m", bf16)
            b_src = b_update(v_sb, first=False)

            # ---------------- iteration 3 (no b-update) ----------------
            if FAST_S3:
                mc_sb = softmax_mc(b_src)
                s_sb = wp.tile([128, BC, NO], f32, name="s_sb", tag="s_sb")
                s_matmul(mc_sb[:], s_sb, scale=None)
            else:
                s_sb = _dekker_s3(b_src)

            rs_in = dp.tile([B, NO], f32, name="rs_in", tag="rs_in")
            rs_out = dp.tile([B_SH, NO], f32, name="rs_out", tag="rs_out")
            for bc_i in range(BC):
                engs[bc_i % 2].dma_start(
                    rs_in[bc_i * 128:(bc_i + 1) * 128, :], s_sb[:, bc_i, :])
            nc.gpsimd.collective_compute(
                "ReduceScatter", ALU.add, replica_groups=RG,
                ins=[rs_in.opt()], outs=[rs_out.opt()])
            ssh = wp.tile([B_SH, 1, NO], f32, name="ssh", tag="ssh")
            nc.sync.dma_start(ssh[:, 0, :], rs_out[:])
            vsh = squash(ssh, B_SH, 1, "s", f32, newton_iters=2)
            nc.sync.dma_start(y_d[:], vsh[:, 0, :])

    nc.compile()
    return nc


def _host_prep(x, W):
    """Per-core input dicts + the constant F matrix."""
    import ml_dtypes

    bf = ml_dtypes.bfloat16
    x = np.ascontiguousarray(x, dtype=np.float32)
    W = np.ascontiguousarray(W, dtype=np.float32)
    F = (np.kron(np.eye(16, dtype=np.float32),
                 np.ones((8, 8), dtype=np.float32)) / np.float32(B)).astype(bf)
    in_maps = []
    for c in range(N_CORES):
        sl = slice(c * I_SH, (c + 1) * I_SH)
        x_sh = x[:, :, sl]                                   # [B, K, I_SH]
        xt = np.ascontiguousarray(x_sh.transpose(2, 1, 0)).reshape(JR, B)
        xt_hi = xt.astype(bf)
        xt_lo = (xt - xt_hi.astype(np.float32)).astype(bf)
        xik = np.ascontiguousarray(
            x_sh.transpose(0, 2, 1)).reshape(B, JR).astype(bf)
        wlf = np.ascontiguousarray(
            (np.float32(0.03) * W[0, sl]).transpose(0, 3, 1, 2)
        ).reshape(JR, NO)
        m = {"xt": xt_hi, "xik": xik, "wl": wlf.astype(bf), "fmat": F}
        if not FAST_S3:
            m["xtl"] = xt_lo
            m["wlf"] = wlf
        in_maps.append(m)
    return in_maps


def _run(in_maps, trace=False, all_cores=False):
    from concourse.bass_utils import run_bass_kernel_spmd

    if "nc" not in _CACHE:
        _CACHE["nc"] = _build_program()
    nc = _CACHE["nc"]
    kwargs = {}
    if all_cores:
        kwargs["trace_cores"] = list(range(N_CORES))
    res = run_bass_kernel_spmd(nc, in_maps, core_ids=list(range(N_CORES)),
                               trace=trace, **kwargs)
    return res


def kernel(x: np.ndarray, W: np.ndarray) -> np.ndarray:
    in_maps = _host_prep(x, W)
    res = _run(in_maps)
    v = np.concatenate([res.results[c]["y"] for c in range(N_CORES)], axis=0)
    return v.reshape(B, N_NODE, O_SZ, 1).astype(np.float32)



# revision 8
# speedup vs baseline: 1.0482x; 1.0455x over previous
"""CapsuleLayer (dynamic routing, 3 iterations) on 8 Trainium2 NeuronCores.

Decomposition (never materializes u_hat = [256,1152,10,16], 189MB):
  - Shard the 1152 input capsules (i) 8 ways: 144 per core.
  - Per-core row space j = (i_local, k), k = in_size = 8 -> 1152 rows
    = 9 chunks of 128 partitions.
  - s_j:  s[b,(n,o)] = sum_j xT[j,b] * (c[j,n] * Wl[j,(n,o)])   (PE matmul,
    contraction over j; Wl = 0.03*W in [(i,k),(n,o)] layout, c broadcast
    over k and o).  Partial over the i-shard -> exchanged across cores.
  - b_ij update via a Gram matrix instead of u_hat:
       Q[j,(n,o)]  = sum_b x[b,j] * v[b,(n,o)]                  (PE matmul)
       pr[j,n]     = sum_o Wl[j,(n,o)] * Q[j,(n,o)]             (DVE+GpSimd)
       uv_rows     = F.T @ pr  per 128-chunk, F = kron(I16, ones8x8)/B
                     (sums over k within each i-group AND replicates the
                     result back to all k-rows, so b stays row-replicated)
  - Iteration 1 uses uniform c = 1/10 (softmax of zeros): s1 = 0.1*(xT.T@Wl).
  - Iterations 1-2 exchange s partials with a bf16 AllGather + on-chip
    3-round tree-reduce (AG is cheaper than AllReduce on this stack).
  - Iteration 3 needs no b-update; the bf16 s3 goes through ReduceScatter
    so each core squashes only its 32-row batch shard; the host just
    concatenates the 8 shards.

Latency plan (the kernel is serial-latency-bound, engines are <20% busy):
  - The 8-core rendezvous barrier releases when the LAST core triggers its
    first collective, so the per-core time-to-first-trigger (input DMA +
    s1 + copy + DMA) is on every core's critical path.  Inputs load via
    few large DMAs spread across engine queues, s1 starts per-chunk.
  - Routing matmuls use bf16 operands (fp32 PE matmuls are 4x slower);
    bf16 keeps the final rel err ~3e-3 against the 2e-2 budget.
  - squash per 128-batch-chunk is pipelined with the Q matmuls of the
    previous chunk; the big W-sized elementwise ops (mc = c*Wl and
    p = Wl*Q) are split DVE/GpSimd.
  - sqrt is a bit-trick + Newton on the DVE so the ScalarE only ever needs
    the Exp activation table (Sqrt/Ln live in other table sets and would
    force ~2.7us ACT_TABLE_LOADs per iteration).
"""
import sys

if "/opt/trn_rl_repo" not in sys.path:
    sys.path.insert(0, "/opt/trn_rl_repo")

import numpy as np

import os
N_CORES = int(os.environ.get("KERNEL_CORES", "8"))
B, IN_SIZE, I_TOT = 256, 8, 1152
N_NODE, O_SZ = 10, 16
NO = N_NODE * O_SZ          # 160
I_SH = I_TOT // N_CORES     # 144 capsules per core
JR = I_SH * IN_SIZE         # 1152 rows per core
NCH = JR // 128             # 9 contraction chunks
BC = B // 128               # 2 batch chunks
B_SH = B // N_CORES         # 32 batch rows per core after ReduceScatter

RSQRT_MAGIC = 0x5F3759DF

_CACHE = {}


def _build_program():
    import concourse.bacc as bacc
    import concourse.tile as tile
    import concourse.mybir as mybir

    f32 = mybir.dt.float32
    bf16 = mybir.dt.bfloat16
    i32 = mybir.dt.int32
    AF = mybir.ActivationFunctionType
    ALU = mybir.AluOpType
    AX = mybir.AxisListType

    nc = bacc.Bacc("TRN2", target_bir_lowering=False, debug=False,
                   enable_asserts=False, num_devices=N_CORES)

    xt_d = nc.dram_tensor("xt", [JR, B], bf16, kind="ExternalInput").ap()
    xik_d = nc.dram_tensor("xik", [B, JR], bf16, kind="ExternalInput").ap()
    wl_d = nc.dram_tensor("wl", [JR, NO], bf16, kind="ExternalInput").ap()
    f_d = nc.dram_tensor("fmat", [128, 128], bf16, kind="ExternalInput").ap()
    y_d = nc.dram_tensor("y", [B_SH, NO], f32, kind="ExternalOutput").ap()

    RG = [list(range(N_CORES))]

    with tile.TileContext(nc) as tc:
        with tc.tile_pool(name="persist", bufs=1) as pp, \
             tc.tile_pool(name="work", bufs=1) as wp, \
             tc.tile_pool(name="ps_s", bufs=2, space="PSUM") as ps_s, \
             tc.tile_pool(name="ps_q", bufs=3, space="PSUM") as ps_q, \
             tc.tile_pool(name="ps_f", bufs=1, space="PSUM") as ps_f, \
             tc.tile_pool(name="dram", bufs=1, space="DRAM") as dp:

            # ---------------- input loads ----------------
            xt_sb = pp.tile([128, NCH, B], bf16, name="xt_sb", tag="xt_sb")
            xik_sb = pp.tile([128, BC, JR], bf16, name="xik_sb", tag="xik_sb")
            wl_sb = pp.tile([128, NCH, NO], bf16, name="wl_sb", tag="wl_sb")
            f_sb = pp.tile([128, 128], bf16, name="f_sb", tag="f_sb")
            b_sb = pp.tile([128, NCH, N_NODE], f32, name="b_sb", tag="b_sb")

            # Few large DMAs, spread across engine DGE queues; s1 needs
            # xt+wl first, xik (Q1) next, F much later.
            xt3 = xt_d.rearrange("(c p) b -> p c b", p=128)
            wl3 = wl_d.rearrange("(c p) f -> p c f", p=128)
            nc.sync.dma_start(xt_sb[:, 0:5, :], xt3[:, 0:5, :])
            nc.scalar.dma_start(wl_sb[:, 0:5, :], wl3[:, 0:5, :])
            nc.sync.dma_start(xt_sb[:, 5:NCH, :], xt3[:, 5:NCH, :])
            nc.scalar.dma_start(wl_sb[:, 5:NCH, :], wl3[:, 5:NCH, :])
            for bc_i in range(BC):
                nc.gpsimd.dma_start(xik_sb[:, bc_i, :],
                                    xik_d[bc_i * 128:(bc_i + 1) * 128, :])
            nc.gpsimd.dma_start(f_sb[:], f_d[:])

            wl4 = wl_sb[:].rearrange("p c (n o) -> p c n o", n=N_NODE)

            # ---------------- helpers ----------------
            def s_matmul(rhs3, s_sb, scale):
                """s_sb[:,bc*NO:] = scale * sum_c xt[:,c,bc].T @ rhs3[:,c,:]
                PSUM->SBUF copies go on scalar (bc0) and vector (bc1) so the
                two convert/copy ops overlap."""
                for bc_i in range(BC):
                    s_ps = ps_s.tile([128, NO], f32, name="s_ps", tag="s_ps")
                    for c in range(NCH):
                        nc.tensor.matmul(
                            s_ps[:],
                            xt_sb[:, c, bc_i * 128:(bc_i + 1) * 128],
                            rhs3[:, c, :],
                            start=(c == 0), stop=(c == NCH - 1))
                    dst = s_sb[:, bc_i * NO:(bc_i + 1) * NO]
                    if bc_i == 0:
                        if scale is None:
                            nc.scalar.copy(dst, s_ps[:])
                        else:
                            nc.scalar.mul(dst, s_ps[:], scale)
                    else:
                        if scale is None:
                            nc.vector.tensor_copy(dst, s_ps[:])
                        else:
                            nc.vector.tensor_scalar_mul(dst, s_ps[:], scale)

            def allgather_s(s_sb, t):
                """AllGather the bf16 s partials and 3-round tree-reduce the
                8 rank partials on the DVE.  Returns sfull [128, BC*NO]."""
                ag_in = dp.tile([128, BC * NO], bf16, name=f"ag_in{t}",
                                tag="ag_in")
                ag_out = dp.tile([N_CORES * 128, BC * NO], bf16,
                                 name=f"ag_out{t}", tag="ag_out")
                nc.sync.dma_start(ag_in[:], s_sb[:])
                nc.gpsimd.collective_compute(
                    "AllGather", ALU.bypass, replica_groups=RG,
                    ins=[ag_in.opt()], outs=[ag_out.opt()])
                agv = wp.tile([128, N_CORES, BC * NO], bf16, name="agv",
                              tag="agv")
                ag3 = ag_out.rearrange("(r p) f -> p r f", p=128)
                nc.sync.dma_start(agv[:, 0:3, :], ag3[:, 0:3, :])
                nc.scalar.dma_start(agv[:, 3:6, :], ag3[:, 3:6, :])
                nc.gpsimd.dma_start(agv[:, 6:8, :], ag3[:, 6:8, :])
                t4 = wp.tile([128, 4, BC * NO], bf16, name="agt4", tag="agt4")
                nc.vector.tensor_add(t4[:], agv[:, 0:4, :], agv[:, 4:8, :])
                t2 = wp.tile([128, 2, BC * NO], bf16, name="agt2", tag="agt2")
                nc.vector.tensor_add(t2[:], t4[:, 0:2, :], t4[:, 2:4, :])
                sfull = wp.tile([128, BC * NO], bf16, name="sfull",
                                tag="sfull")
                nc.vector.tensor_add(sfull[:], t2[:, 0, :], t2[:, 1, :])
                return sfull

            def rsqrt(msq, P, nch, tag, iters):
                """z ~ 1/sqrt(msq) via int bit-trick + Newton steps (DVE
                only -- avoids the Sqrt/Ln ACT table sets entirely)."""
                sh = [P, nch, N_NODE]
                zi = wp.tile(sh, i32, name="zi" + tag, tag="zi" + tag)
                nc.vector.tensor_scalar(
                    out=zi[:], in0=msq[:].bitcast(i32), scalar1=1, scalar2=-1,
                    op0=ALU.arith_shift_right, op1=ALU.bitwise_xor)
                nc.vector.tensor_scalar_add(zi[:], zi[:], RSQRT_MAGIC + 1)
                z = zi[:].bitcast(f32)
                t = wp.tile(sh, f32, name="nt" + tag, tag="nt" + tag)
                w = wp.tile(sh, f32, name="nw" + tag, tag="nw" + tag)
                for _ in range(iters):
                    nc.vector.tensor_mul(t[:], z, z)
                    nc.vector.tensor_mul(t[:], t[:], msq[:])
                    nc.vector.tensor_scalar(
                        out=w[:], in0=t[:], scalar1=-0.5, scalar2=1.5,
                        op0=ALU.mult, op1=ALU.add)
                    nc.vector.tensor_mul(z, z, w[:])
                return z

            def squash(s_ap, P, nch, tag, v_dtype, newton_iters=1, v_sb=None,
                       v_off=0):
                """v = squash(s) over o. s_ap [P, nch, NO].  One Newton step
                (~0.2% rsqrt error) suffices for the routing iterations; the
                output pass uses two (~1e-5)."""
                s4 = s_ap.rearrange("p c (n o) -> p c n o", n=N_NODE)
                sq = wp.tile([P, nch, NO], f32, name="sq" + tag,
                             tag="sq" + tag)
                nc.vector.tensor_mul(sq[:], s_ap, s_ap)
                msq = wp.tile([P, nch, N_NODE], f32, name="msq" + tag,
                              tag="msq" + tag)
                nc.vector.reduce_sum(
                    msq[:], sq[:].rearrange("p c (n o) -> p c n o", n=N_NODE),
                    axis=AX.X)
                den = wp.tile([P, nch, N_NODE], f32, name="den" + tag,
                              tag="den" + tag)
                nc.vector.tensor_scalar_add(den[:], msq[:], 1.0)
                rden = wp.tile([P, nch, N_NODE], f32, name="rden" + tag,
                               tag="rden" + tag)
                nc.vector.reciprocal(rden[:], den[:])
                z = rsqrt(msq, P, nch, tag, newton_iters)
                mag = wp.tile([P, nch, N_NODE], f32, name="mag" + tag,
                              tag="mag" + tag)
                nc.vector.tensor_mul(mag[:], msq[:], z)   # sqrt(msq)
                fac = wp.tile([P, nch, N_NODE], f32, name="fac" + tag,
                              tag="fac" + tag)
                nc.vector.tensor_mul(fac[:], mag[:], rden[:])
                if v_sb is None:
                    v_sb = wp.tile([P, nch, NO], v_dtype, name="v" + tag,
                                   tag="v" + tag)
                    v4 = v_sb[:].rearrange("p c (n o) -> p c n o", n=N_NODE)
                else:
                    v4 = v_sb[:, v_off:v_off + nch, :].rearrange(
                        "p c (n o) -> p c n o", n=N_NODE)
                fb = fac[:].unsqueeze(3).broadcast_to((P, nch, N_NODE, O_SZ))
                nc.vector.tensor_mul(v4, s4, fb)
                return v_sb

            def squash_and_q(sfull):
                """Squash per 128-batch-chunk, pipelined with the Q matmuls
                (Q accumulates over the two batch chunks; the PE starts on
                chunk 0 while the DVE squashes chunk 1).  Then
                p = wl*Q (split DVE/GpSimd), pr = sum_o p."""
                sf3 = sfull[:].rearrange("p (c f) -> p c f", c=BC)
                v_sb = wp.tile([128, BC, NO], bf16, name="v_m", tag="v_m")
                q_tiles = []
                for g in range(NCH // 3):
                    q_tiles.append(ps_q.tile([128, 3 * NO], f32, name="q_ps",
                                             tag="q_ps"))
                for bc_i in range(BC):
                    squash(sf3[:, bc_i:bc_i + 1, :], 128, 1, "m",
                           bf16, v_sb=v_sb, v_off=bc_i)
                    for g in range(NCH // 3):
                        for s_i in range(3):
                            mc = g * 3 + s_i
                            nc.tensor.matmul(
                                q_tiles[g][:, s_i * NO:(s_i + 1) * NO],
                                xik_sb[:, bc_i, mc * 128:(mc + 1) * 128],
                                v_sb[:, bc_i, :],
                                start=(bc_i == 0), stop=(bc_i == BC - 1))
                # p = wl*Q must stay on DVE (only DVE reads PSUM for
                # tensor-tensor); per-group so it pipelines with the MMs.
                p_sb = wp.tile([128, NCH, NO], bf16, name="p_sb", tag="p_sb")
                pr = wp.tile([128, NCH, N_NODE], f32, name="pr_sb",
                             tag="pr_sb")
                for g in range(NCH // 3):
                    q3 = q_tiles[g][:].rearrange("p (c f) -> p c f", c=3)
                    nc.vector.tensor_mul(
                        p_sb[:, g * 3:(g + 1) * 3, :],
                        wl_sb[:, g * 3:(g + 1) * 3, :], q3[:])
                    nc.vector.reduce_sum(
                        pr[:, g * 3:(g + 1) * 3, :],
                        p_sb[:, g * 3:(g + 1) * 3, :].rearrange(
                            "p c (n o) -> p c n o", n=N_NODE),
                        axis=AX.X)
                return v_sb, pr

            def b_update(pr, first):
                prb = wp.tile([128, NCH, N_NODE], bf16, name="prb", tag="prb")
                nc.vector.tensor_copy(prb[:], pr[:])
                uv_ps = ps_f.tile([128, NCH * N_NODE], f32, name="uv_ps",
                                  tag="uv_ps")
                nc.tensor.matmul(uv_ps[:], f_sb[:],
                                 prb[:].rearrange("p c n -> p (c n)"),
                                 start=True, stop=True)
                uv3 = uv_ps[:].rearrange("p (c n) -> p c n", n=N_NODE)
                if first:
                    # keep b state for the next update, but let the softmax
                    # read the PSUM uv directly (shorter critical path)
                    nc.scalar.copy(b_sb[:], uv3)
                    return uv3
                nc.vector.tensor_add(b_sb[:], b_sb[:], uv3)
                return b_sb[:]

            def softmax_mc(b_src):
                e_sb = wp.tile([128, NCH, N_NODE], f32, name="e_sb",
                               tag="e_sb")
                nc.scalar.activation(e_sb[:], b_src, AF.Exp)
                se = wp.tile([128, NCH], f32, name="se", tag="se")
                nc.vector.reduce_sum(se[:], e_sb[:], axis=AX.X)
                rse = wp.tile([128, NCH], f32, name="rse", tag="rse")
                nc.vector.reciprocal(rse[:], se[:])
                c_sb = wp.tile([128, NCH, N_NODE], bf16, name="c_sb",
                               tag="c_sb")
                nc.vector.tensor_mul(
                    c_sb[:], e_sb[:],
                    rse[:].unsqueeze(2).broadcast_to((128, NCH, N_NODE)))
                mc_sb = wp.tile([128, NCH, NO], bf16, name="mc_sb",
                                tag="mc_sb")
                cb = c_sb[:].unsqueeze(3).broadcast_to(
                    (128, NCH, N_NODE, O_SZ))
                mc4 = mc_sb[:].rearrange("p c (n o) -> p c n o", n=N_NODE)
                # split the W-sized multiply across DVE and the idle GpSimd
                nc.vector.tensor_mul(mc4[:, 0:7], wl4[:, 0:7], cb[:, 0:7])
                nc.gpsimd.tensor_mul(mc4[:, 7:NCH], wl4[:, 7:NCH],
                                     cb[:, 7:NCH])
                return mc_sb

            # ---------------- iteration 1 (c uniform = 0.1) ----------------
            s_sb = wp.tile([128, BC * NO], bf16, name="s_sbr", tag="s_sbr")
            s_matmul(wl_sb[:], s_sb, scale=0.1)
            sfull = allgather_s(s_sb, 0)
            v_sb, pr = squash_and_q(sfull)
            b_src = b_update(pr, first=True)

            # ---------------- iteration 2 ----------------
            mc_sb = softmax_mc(b_src)
            s_sb = wp.tile([128, BC * NO], bf16, name="s_sbr", tag="s_sbr")
            s_matmul(mc_sb[:], s_sb, scale=None)
            sfull = allgather_s(s_sb, 1)
            v_sb, pr = squash_and_q(sfull)
            b_src = b_update(pr, first=False)

            # ---------------- iteration 3 (no b-update) ----------------
            mc_sb = softmax_mc(b_src)
            s_sb = wp.tile([128, BC * NO], bf16, name="s_sbr", tag="s_sbr")
            s_matmul(mc_sb[:], s_sb, scale=None)

            rs_in = dp.tile([B, NO], bf16, name="rs_in", tag="rs_in")
            rs_out = dp.tile([B_SH, NO], bf16, name="rs_out", tag="rs_out")
            s2 = s_sb[:].rearrange("p (c f) -> p c f", c=BC)
            for bc_i in range(BC):
                eng = nc.sync if bc_i == 0 else nc.scalar
                eng.dma_start(rs_in[bc_i * 128:(bc_i + 1) * 128, :],
                              s2[:, bc_i, :])
            nc.gpsimd.collective_compute(
                "ReduceScatter", ALU.add, replica_groups=RG,
                ins=[rs_in.opt()], outs=[rs_out.opt()])
            ssh = wp.tile([B_SH, 1, NO], bf16, name="ssh", tag="ssh")
            nc.sync.dma_start(ssh[:, 0, :], rs_out[:])
            vsh = squash(ssh[:], B_SH, 1, "s", f32, newton_iters=2)
            nc.sync.dma_start(y_d[:], vsh[:, 0, :])

    nc.compile()
    return nc


def _host_prep(x, W):
    """Per-core input dicts + the constant F matrix."""
    import ml_dtypes

    bf = ml_dtypes.bfloat16
    x = np.ascontiguousarray(x, dtype=np.float32)
    W = np.ascontiguousarray(W, dtype=np.float32)
    F = (np.kron(np.eye(16, dtype=np.float32),
                 np.ones((8, 8), dtype=np.float32)) / np.float32(B)).astype(bf)
    in_maps = []
    for c in range(N_CORES):
        sl = slice(c * I_SH, (c + 1) * I_SH)
        x_sh = x[:, :, sl]                                   # [B, K, I_SH]
        xt = np.ascontiguousarray(x_sh.transpose(2, 1, 0)).reshape(JR, B)
        xik = np.ascontiguousarray(
            x_sh.transpose(0, 2, 1)).reshape(B, JR).astype(bf)
        wlf = np.ascontiguousarray(
            (np.float32(0.03) * W[0, sl]).transpose(0, 3, 1, 2)
        ).reshape(JR, NO)
        m = {"xt": xt.astype(bf), "xik": xik, "wl": wlf.astype(bf),
             "fmat": F}
        in_maps.append(m)
    return in_maps


def _run(in_maps, trace=False, all_cores=False):
    from concourse.bass_utils import run_bass_kernel_spmd

    if "nc" not in _CACHE:
        _CACHE["nc"] = _build_program()
    nc = _CACHE["nc"]
    kwargs = {}
    if all_cores:
        kwargs["trace_cores"] = list(range(N_CORES))
    res = run_bass_kernel_spmd(nc, in_maps, core_ids=list(range(N_CORES)),
                               trace=trace, **kwargs)
    return res


def kernel(x: np.ndarray, W: np.ndarray) -> np.ndarray:
    in_maps = _host_prep(x, W)
    res = _run(in_maps)
    v = np.concatenate([res.results[c]["y"] for c in range(N_CORES)], axis=0)
    return v.reshape(B, N_NODE, O_SZ, 1).astype(np.float32)


# revision 13
# speedup vs baseline: 1.0538x; 1.0053x over previous
"""CapsuleLayer (dynamic routing, 3 iterations) on 8 Trainium2 NeuronCores.

Decomposition (never materializes u_hat = [256,1152,10,16], 189MB):
  - Shard the 1152 input capsules (i) 8 ways: 144 per core.
  - Per-core row space j = (i_local, k), k = in_size = 8 -> 1152 rows
    = 9 chunks of 128 partitions.
  - s_j:  s[b,(n,o)] = sum_j xT[j,b] * (c[j,n] * Wl[j,(n,o)])   (PE matmul,
    contraction over j; Wl = 0.03*W in [(i,k),(n,o)] layout, c broadcast
    over k and o).  Partial over the i-shard -> summed across cores by a
    fp32 AllReduce straight out of PSUM (no SBUF copies, no on-chip tree).
  - b_ij update via a Gram matrix instead of u_hat:
       Q[j,(n,o)]  = sum_b x[b,j] * v[b,(n,o)]                  (PE matmul)
       pr[j,n]     = sum_o Wl[j,(n,o)] * Q[j,(n,o)]             (DVE)
       uv_rows     = F.T @ pr  per 128-chunk, F = kron(I16, ones8x8)/B
                     (sums over k within each i-group AND replicates the
                     result back to all k-rows, so b stays row-replicated)
  - Iteration 1 uses uniform c = 1/10 (softmax of zeros): s1 = 0.1*(xT.T@Wl).
  - Iteration 3 needs no b-update; the fp32 s3 goes through ReduceScatter
    (also straight out of PSUM) so each core squashes only its 32-row
    batch shard; the host just concatenates the 8 shards.

Latency plan (the kernel is serial-latency-bound, engines are <20% busy):
  - The 8-core rendezvous barrier releases when the LAST core triggers its
    first collective, so per-core time-to-first-trigger is on every core's
    critical path.  All inputs are host-transposed to partition-major so
    each loads with ONE plain 2D contiguous DMA (3D-strided dma_starts
    cost ~1.3-1.8us of descriptor generation each; 2D cost ~0.6us).
  - Routing matmuls use bf16 operands (fp32 PE matmuls are 4x slower).
  - squash per 128-batch-chunk is pipelined with the Q matmuls of the
    previous chunk; the W-sized mc = c*Wl multiply is split DVE/GpSimd.
  - sqrt is a bit-trick + Newton on the DVE so the ScalarE only ever needs
    the Exp activation table (Sqrt/Ln live in other table sets and would
    force ~2.7us ACT_TABLE_LOADs per iteration).  The routing squashes
    skip the Newton step entirely (~3.4% rsqrt error, which averages out
    across the 1152-capsule contraction); the output squash uses two.
"""
import sys

if "/opt/trn_rl_repo" not in sys.path:
    sys.path.insert(0, "/opt/trn_rl_repo")

import numpy as np

import os
N_CORES = int(os.environ.get("KERNEL_CORES", "8"))
B, IN_SIZE, I_TOT = 256, 8, 1152
N_NODE, O_SZ = 10, 16
NO = N_NODE * O_SZ          # 160
I_SH = I_TOT // N_CORES     # 144 capsules per core
JR = I_SH * IN_SIZE         # 1152 rows per core
NCH = JR // 128             # 9 contraction chunks
BC = B // 128               # 2 batch chunks
B_SH = B // N_CORES         # 32 batch rows per core after ReduceScatter

RSQRT_MAGIC = 0x5F3759DF
NEWTON_ROUTE = int(os.environ.get("KERNEL_NEWTON_ROUTE", "0"))

_CACHE = {}


def _build_program():
    import concourse.bacc as bacc
    import concourse.tile as tile
    import concourse.mybir as mybir

    f32 = mybir.dt.float32
    bf16 = mybir.dt.bfloat16
    i32 = mybir.dt.int32
    AF = mybir.ActivationFunctionType
    ALU = mybir.AluOpType
    AX = mybir.AxisListType

    nc = bacc.Bacc("TRN2", target_bir_lowering=False, debug=False,
                   enable_asserts=False, num_devices=N_CORES)

    # All inputs partition-major: one plain 2D contiguous DMA each.
    xt_d = nc.dram_tensor("xt", [128, NCH * B], bf16,
                          kind="ExternalInput").ap()
    xik_d = nc.dram_tensor("xik", [128, BC * JR], bf16,
                           kind="ExternalInput").ap()
    wl_d = nc.dram_tensor("wl", [128, NCH * NO], bf16,
                          kind="ExternalInput").ap()
    f_d = nc.dram_tensor("fmat", [128, 128], bf16, kind="ExternalInput").ap()
    y_d = nc.dram_tensor("y", [B_SH, NO], f32, kind="ExternalOutput").ap()

    RG = [list(range(N_CORES))]

    with tile.TileContext(nc) as tc:
        with tc.tile_pool(name="persist", bufs=1) as pp, \
             tc.tile_pool(name="work", bufs=1) as wp, \
             tc.tile_pool(name="ps_s", bufs=2, space="PSUM") as ps_s, \
             tc.tile_pool(name="ps_q", bufs=3, space="PSUM") as ps_q, \
             tc.tile_pool(name="ps_f", bufs=1, space="PSUM") as ps_f, \
             tc.tile_pool(name="dram", bufs=1, space="DRAM") as dp:

            # ---------------- input loads ----------------
            xt_sb = pp.tile([128, NCH, B], bf16, name="xt_sb", tag="xt_sb")
            xik_sb = pp.tile([128, BC, JR], bf16, name="xik_sb", tag="xik_sb")
            wl_sb = pp.tile([128, NCH, NO], bf16, name="wl_sb", tag="wl_sb")
            f_sb = pp.tile([128, 128], bf16, name="f_sb", tag="f_sb")
            b_sb = pp.tile([128, NCH, N_NODE], f32, name="b_sb", tag="b_sb")

            nc.sync.dma_start(
                xt_sb[:].rearrange("p c b -> p (c b)"), xt_d[:])
            nc.scalar.dma_start(
                wl_sb[:].rearrange("p c f -> p (c f)"), wl_d[:])
            nc.gpsimd.dma_start(
                xik_sb[:].rearrange("p c j -> p (c j)"), xik_d[:])
            nc.gpsimd.dma_start(f_sb[:], f_d[:])

            wl4 = wl_sb[:].rearrange("p c (n o) -> p c n o", n=N_NODE)

            # ---------------- helpers ----------------
            def s_matmul(rhs3, ar_dsts):
                """ar_dsts[bc] (DRAM) = sum_c xt[:,c,bc].T @ rhs3[:,c,:]
                per batch-chunk: bc0's PSUM->SBUF copy + store DMA overlap
                bc1's matmuls (DMA cannot source PSUM directly)."""
                s_sb = wp.tile([128, BC, NO], f32, name="s_st", tag="s_st")
                for bc_i in range(BC):
                    s_ps = ps_s.tile([128, NO], f32, name="s_ps", tag="s_ps")
                    for c in range(NCH):
                        nc.tensor.matmul(
                            s_ps[:],
                            xt_sb[:, c, bc_i * 128:(bc_i + 1) * 128],
                            rhs3[:, c, :],
                            start=(c == 0), stop=(c == NCH - 1))
                    if bc_i == 0:
                        nc.scalar.copy(s_sb[:, 0, :], s_ps[:])
                        nc.sync.dma_start(ar_dsts[0], s_sb[:, 0, :])
                    else:
                        nc.vector.tensor_copy(s_sb[:, 1, :], s_ps[:])
                        nc.scalar.dma_start(ar_dsts[1], s_sb[:, 1, :])

            def allreduce_s(t):
                ar_in = dp.tile([128, BC * NO], f32, name=f"ar_in{t}",
                                tag="ar_in")
                ar_out = dp.tile([128, BC * NO], f32, name=f"ar_out{t}",
                                 tag="ar_out")
                return ar_in, ar_out

            def rsqrt(msq, P, nch, tag, iters):
                """z ~ 1/sqrt(msq) via int bit-trick + Newton steps (DVE
                only -- avoids the Sqrt/Ln ACT table sets entirely)."""
                sh = [P, nch, N_NODE]
                zi = wp.tile(sh, i32, name="zi" + tag, tag="zi" + tag)
                nc.vector.tensor_scalar(
                    out=zi[:], in0=msq[:].bitcast(i32), scalar1=1, scalar2=-1,
                    op0=ALU.arith_shift_right, op1=ALU.bitwise_xor)
                nc.vector.tensor_scalar_add(zi[:], zi[:], RSQRT_MAGIC + 1)
                z = zi[:].bitcast(f32)
                t = wp.tile(sh, f32, name="nt" + tag, tag="nt" + tag)
                w = wp.tile(sh, f32, name="nw" + tag, tag="nw" + tag)
                for _ in range(iters):
                    nc.vector.tensor_mul(t[:], z, z)
                    nc.vector.tensor_mul(t[:], t[:], msq[:])
                    nc.vector.tensor_scalar(
                        out=w[:], in0=t[:], scalar1=-0.5, scalar2=1.5,
                        op0=ALU.mult, op1=ALU.add)
                    nc.vector.tensor_mul(z, z, w[:])
                return z

            def squash(s_ap, P, nch, tag, v_dtype, newton_iters, v_sb=None,
                       v_off=0, scale=None):
                """v = squash(s * scale) over o.  s_ap [P, nch, NO]."""
                s4 = s_ap.rearrange("p c (n o) -> p c n o", n=N_NODE)
                sq = wp.tile([P, nch, NO], f32, name="sq" + tag,
                             tag="sq" + tag)
                nc.vector.tensor_mul(sq[:], s_ap, s_ap)
                msq = wp.tile([P, nch, N_NODE], f32, name="msq" + tag,
                              tag="msq" + tag)
                nc.vector.reduce_sum(
                    msq[:], sq[:].rearrange("p c (n o) -> p c n o", n=N_NODE),
                    axis=AX.X)
                if scale is not None:
                    # s was pre-scale; msq *= scale^2 so fac comes out right,
                    # and the final v-mul absorbs scale via fac*scale.
                    nc.vector.tensor_scalar_mul(msq[:], msq[:],
                                                float(scale * scale))
                den = wp.tile([P, nch, N_NODE], f32, name="den" + tag,
                              tag="den" + tag)
                nc.vector.tensor_scalar_add(den[:], msq[:], 1.0)
                rden = wp.tile([P, nch, N_NODE], f32, name="rden" + tag,
                               tag="rden" + tag)
                nc.vector.reciprocal(rden[:], den[:])
                z = rsqrt(msq, P, nch, tag, newton_iters)
                mag = wp.tile([P, nch, N_NODE], f32, name="mag" + tag,
                              tag="mag" + tag)
                nc.vector.tensor_mul(mag[:], msq[:], z)   # sqrt(msq)
                fac = wp.tile([P, nch, N_NODE], f32, name="fac" + tag,
                              tag="fac" + tag)
                nc.vector.tensor_mul(fac[:], mag[:], rden[:])
                if scale is not None:
                    nc.vector.tensor_scalar_mul(fac[:], fac[:], float(scale))
                if v_sb is None:
                    v_sb = wp.tile([P, nch, NO], v_dtype, name="v" + tag,
                                   tag="v" + tag)
                    v4 = v_sb[:].rearrange("p c (n o) -> p c n o", n=N_NODE)
                else:
                    v4 = v_sb[:, v_off:v_off + nch, :].rearrange(
                        "p c (n o) -> p c n o", n=N_NODE)
                fb = fac[:].unsqueeze(3).broadcast_to((P, nch, N_NODE, O_SZ))
                nc.vector.tensor_mul(v4, s4, fb)
                return v_sb

            def squash_and_q(ar_out, scale=None):
                """DMA the AllReduced s back per batch-chunk, squash each,
                pipelined with the Q matmuls; then p = wl*Q on the DVE
                (only DVE can read PSUM for tensor-tensor), per PSUM group
                so it pipelines with the remaining matmuls."""
                sf = wp.tile([128, BC, NO], f32, name="sf", tag="sf")
                for bc_i in range(BC):
                    eng = nc.sync if bc_i == 0 else nc.scalar
                    eng.dma_start(sf[:, bc_i, :],
                                  ar_out[:, bc_i * NO:(bc_i + 1) * NO])
                v_sb = wp.tile([128, BC, NO], bf16, name="v_m", tag="v_m")
                q_tiles = []
                for g in range(NCH // 3):
                    q_tiles.append(ps_q.tile([128, 3 * NO], f32, name="q_ps",
                                             tag="q_ps"))
                for bc_i in range(BC):
                    squash(sf[:, bc_i:bc_i + 1, :], 128, 1, "m",
                           bf16, NEWTON_ROUTE, v_sb=v_sb, v_off=bc_i,
                           scale=scale)
                    for g in range(NCH // 3):
                        for s_i in range(3):
                            mc = g * 3 + s_i
                            nc.tensor.matmul(
                                q_tiles[g][:, s_i * NO:(s_i + 1) * NO],
                                xik_sb[:, bc_i, mc * 128:(mc + 1) * 128],
                                v_sb[:, bc_i, :],
                                start=(bc_i == 0), stop=(bc_i == BC - 1))
                p_sb = wp.tile([128, NCH, NO], bf16, name="p_sb", tag="p_sb")
                pr = wp.tile([128, NCH, N_NODE], f32, name="pr_sb",
                             tag="pr_sb")
                for g in range(NCH // 3):
                    q3 = q_tiles[g][:].rearrange("p (c f) -> p c f", c=3)
                    nc.vector.tensor_mul(
                        p_sb[:, g * 3:(g + 1) * 3, :],
                        wl_sb[:, g * 3:(g + 1) * 3, :], q3[:])
                    nc.vector.reduce_sum(
                        pr[:, g * 3:(g + 1) * 3, :],
                        p_sb[:, g * 3:(g + 1) * 3, :].rearrange(
                            "p c (n o) -> p c n o", n=N_NODE),
                        axis=AX.X)
                return v_sb, pr

            def b_update(pr, first):
                prb = wp.tile([128, NCH, N_NODE], bf16, name="prb", tag="prb")
                nc.vector.tensor_copy(prb[:], pr[:])
                uv_ps = ps_f.tile([128, NCH * N_NODE], f32, name="uv_ps",
                                  tag="uv_ps")
                nc.tensor.matmul(uv_ps[:], f_sb[:],
                                 prb[:].rearrange("p c n -> p (c n)"),
                                 start=True, stop=True)
                uv3 = uv_ps[:].rearrange("p (c n) -> p c n", n=N_NODE)
                if first:
                    # keep b state for the next update, but let the softmax
                    # read the PSUM uv directly (shorter critical path)
                    nc.scalar.copy(b_sb[:], uv3)
                    return uv3
                nc.vector.tensor_add(b_sb[:], b_sb[:], uv3)
                return b_sb[:]

            def softmax_mc(b_src):
                e_sb = wp.tile([128, NCH, N_NODE], f32, name="e_sb",
                               tag="e_sb")
                nc.scalar.activation(e_sb[:], b_src, AF.Exp)
                se = wp.tile([128, NCH], f32, name="se", tag="se")
                nc.vector.reduce_sum(se[:], e_sb[:], axis=AX.X)
                rse = wp.tile([128, NCH], f32, name="rse", tag="rse")
                nc.vector.reciprocal(rse[:], se[:])
                c_sb = wp.tile([128, NCH, N_NODE], bf16, name="c_sb",
                               tag="c_sb")
                nc.vector.tensor_mul(
                    c_sb[:], e_sb[:],
                    rse[:].unsqueeze(2).broadcast_to((128, NCH, N_NODE)))
                mc_sb = wp.tile([128, NCH, NO], bf16, name="mc_sb",
                                tag="mc_sb")
                cb = c_sb[:].unsqueeze(3).broadcast_to(
                    (128, NCH, N_NODE, O_SZ))
                mc4 = mc_sb[:].rearrange("p c (n o) -> p c n o", n=N_NODE)
                # split the W-sized multiply across DVE and the idle GpSimd
                nc.vector.tensor_mul(mc4[:, 0:8], wl4[:, 0:8], cb[:, 0:8])
                nc.gpsimd.tensor_mul(mc4[:, 8:NCH], wl4[:, 8:NCH],
                                     cb[:, 8:NCH])
                return mc_sb

            # -------- iteration 1 (c uniform = 0.1, folded into squash) ----
            ar_in, ar_out = allreduce_s(0)
            s_matmul(wl_sb[:], [ar_in[:, 0:NO], ar_in[:, NO:2 * NO]])
            nc.gpsimd.collective_compute(
                "AllReduce", ALU.add, replica_groups=RG,
                ins=[ar_in.opt()], outs=[ar_out.opt()])
            v_sb, pr = squash_and_q(ar_out, scale=0.1)
            b_src = b_update(pr, first=True)

            # ---------------- iteration 2 ----------------
            mc_sb = softmax_mc(b_src)
            ar_in, ar_out = allreduce_s(1)
            s_matmul(mc_sb[:], [ar_in[:, 0:NO], ar_in[:, NO:2 * NO]])
            nc.gpsimd.collective_compute(
                "AllReduce", ALU.add, replica_groups=RG,
                ins=[ar_in.opt()], outs=[ar_out.opt()])
            v_sb, pr = squash_and_q(ar_out)
            b_src = b_update(pr, first=False)

            # ---------------- iteration 3 (no b-update) ----------------
            mc_sb = softmax_mc(b_src)
            rs_in = dp.tile([B, NO], f32, name="rs_in", tag="rs_in")
            rs_out = dp.tile([B_SH, NO], f32, name="rs_out", tag="rs_out")
            rs2 = rs_in.rearrange("(c p) f -> p c f", p=128)
            s_matmul(mc_sb[:], [rs2[:, 0, :], rs2[:, 1, :]])
            nc.gpsimd.collective_compute(
                "ReduceScatter", ALU.add, replica_groups=RG,
                ins=[rs_in.opt()], outs=[rs_out.opt()])
            ssh = wp.tile([B_SH, 1, NO], f32, name="ssh", tag="ssh")
            nc.sync.dma_start(ssh[:, 0, :], rs_out[:])
            vsh = squash(ssh[:], B_SH, 1, "s", f32, 2)
            nc.sync.dma_start(y_d[:], vsh[:, 0, :])

    nc.compile()
    return nc


def _host_prep(x, W):
    """Per-core input dicts (partition-major layouts) + the F matrix."""
    import ml_dtypes

    bf = ml_dtypes.bfloat16
    x = np.ascontiguousarray(x, dtype=np.float32)
    W = np.ascontiguousarray(W, dtype=np.float32)
    F = (np.kron(np.eye(16, dtype=np.float32),
                 np.ones((8, 8), dtype=np.float32)) / np.float32(B)).astype(bf)
    in_maps = []
    for c in range(N_CORES):
        sl = slice(c * I_SH, (c + 1) * I_SH)
        x_sh = x[:, :, sl]                                   # [B, K, I_SH]
        # xt rows j=(i,k): [JR, B] -> partition-major [128, NCH, B]
        xt = np.ascontiguousarray(x_sh.transpose(2, 1, 0)).reshape(JR, B)
        xt_pm = np.ascontiguousarray(
            xt.reshape(NCH, 128, B).transpose(1, 0, 2)).reshape(128, NCH * B)
        # xik [B, JR] -> [128, BC, JR]
        xik = np.ascontiguousarray(
            x_sh.transpose(0, 2, 1)).reshape(B, JR)
        xik_pm = np.ascontiguousarray(
            xik.reshape(BC, 128, JR).transpose(1, 0, 2)).reshape(
                128, BC * JR)
        # wl rows j: [JR, NO] -> [128, NCH, NO]
        wlf = np.ascontiguousarray(
            (np.float32(0.03) * W[0, sl]).transpose(0, 3, 1, 2)
        ).reshape(JR, NO)
        wl_pm = np.ascontiguousarray(
            wlf.reshape(NCH, 128, NO).transpose(1, 0, 2)).reshape(
                128, NCH * NO)
        m = {"xt": xt_pm.astype(bf), "xik": xik_pm.astype(bf),
             "wl": wl_pm.astype(bf), "fmat": F}
        in_maps.append(m)
    return in_maps


def _run(in_maps, trace=False, all_cores=False):
    from concourse.bass_utils import run_bass_kernel_spmd

    if "nc" not in _CACHE:
        _CACHE["nc"] = _build_program()
    nc = _CACHE["nc"]
    kwargs = {}
    if all_cores:
        kwargs["trace_cores"] = list(range(N_CORES))
    res = run_bass_kernel_spmd(nc, in_maps, core_ids=list(range(N_CORES)),
                               trace=trace, **kwargs)
    return res


def kernel(x: np.ndarray, W: np.ndarray) -> np.ndarray:
    in_maps = _host_prep(x, W)
    res = _run(in_maps)
    v = np.concatenate([res.results[c]["y"] for c in range(N_CORES)], axis=0)
    return v.reshape(B, N_NODE, O_SZ, 1).astype(np.float32)


# revision 16
# speedup vs baseline: 1.1406x; 1.0824x over previous
"""CapsuleLayer (dynamic routing, 3 iterations) on 8 Trainium2 NeuronCores.

Decomposition (never materializes u_hat = [256,1152,10,16], 189MB):
  - Shard the 1152 input capsules (i) 8 ways: 144 per core.
  - Per-core row space j = (i_local, k), k = in_size = 8 -> 1152 rows
    = 9 chunks of 128 partitions.
  - s_j:  s[b,(n,o)] = sum_j xT[j,b] * (c[j,n] * Wl[j,(n,o)])   (PE matmul,
    contraction over j; Wl = 0.03*W in [(i,k),(n,o)] layout, c broadcast
    over k and o).  Partial over the i-shard -> summed across cores by a
    fp32 AllReduce straight out of PSUM (no SBUF copies, no on-chip tree).
  - b_ij update via a Gram matrix instead of u_hat:
       Q[j,(n,o)]  = sum_b x[b,j] * v[b,(n,o)]                  (PE matmul)
       pr[j,n]     = sum_o Wl[j,(n,o)] * Q[j,(n,o)]             (DVE)
       uv_rows     = F.T @ pr  per 128-chunk, F = kron(I16, ones8x8)/B
                     (sums over k within each i-group AND replicates the
                     result back to all k-rows, so b stays row-replicated)
  - Iteration 1 uses uniform c = 1/10 (softmax of zeros): s1 = 0.1*(xT.T@Wl).
  - Iteration 3 needs no b-update; the fp32 s3 goes through ReduceScatter
    (also straight out of PSUM) so each core squashes only its 32-row
    batch shard; the host just concatenates the 8 shards.

Latency plan (the kernel is serial-latency-bound, engines are <20% busy):
  - The 8-core rendezvous barrier releases when the LAST core triggers its
    first collective, so per-core time-to-first-trigger is on every core's
    critical path.  All inputs are host-transposed to partition-major so
    each loads with ONE plain 2D contiguous DMA (3D-strided dma_starts
    cost ~1.3-1.8us of descriptor generation each; 2D cost ~0.6us).
  - Routing matmuls use bf16 operands (fp32 PE matmuls are 4x slower).
  - squash per 128-batch-chunk is pipelined with the Q matmuls of the
    previous chunk; the W-sized mc = c*Wl multiply is split DVE/GpSimd.
  - sqrt is a bit-trick + Newton on the DVE so the ScalarE only ever needs
    the Exp activation table (Sqrt/Ln live in other table sets and would
    force ~2.7us ACT_TABLE_LOADs per iteration).  The routing squashes
    skip the Newton step entirely (~3.4% rsqrt error, which averages out
    across the 1152-capsule contraction); the output squash uses two.
"""
import sys

if "/opt/trn_rl_repo" not in sys.path:
    sys.path.insert(0, "/opt/trn_rl_repo")

import numpy as np

import os
N_CORES = int(os.environ.get("KERNEL_CORES", "8"))
B, IN_SIZE, I_TOT = 256, 8, 1152
N_NODE, O_SZ = 10, 16
NO = N_NODE * O_SZ          # 160
I_SH = I_TOT // N_CORES     # 144 capsules per core
JR = I_SH * IN_SIZE         # 1152 rows per core
NCH = JR // 128             # 9 contraction chunks
BC = B // 128               # 2 batch chunks
B_SH = B // N_CORES         # 32 batch rows per core after ReduceScatter

RSQRT_MAGIC = 0x5F3759DF
NEWTON_ROUTE = int(os.environ.get("KERNEL_NEWTON_ROUTE", "0"))
EXCHANGE = os.environ.get("KERNEL_EXCHANGE", "ag8")

_CACHE = {}


def _build_program():
    import concourse.bacc as bacc
    import concourse.tile as tile
    import concourse.mybir as mybir

    f32 = mybir.dt.float32
    bf16 = mybir.dt.bfloat16
    f8 = mybir.dt.float8e4
    i32 = mybir.dt.int32
    AF = mybir.ActivationFunctionType
    ALU = mybir.AluOpType
    AX = mybir.AxisListType

    nc = bacc.Bacc("TRN2", target_bir_lowering=False, debug=False,
                   enable_asserts=False, num_devices=N_CORES)

    # All inputs partition-major: one plain 2D contiguous DMA each.
    xt_d = nc.dram_tensor("xt", [128, NCH * B], bf16,
                          kind="ExternalInput").ap()
    xik_d = nc.dram_tensor("xik", [128, BC * JR], bf16,
                           kind="ExternalInput").ap()
    wl_d = nc.dram_tensor("wl", [128, NCH * NO], bf16,
                          kind="ExternalInput").ap()
    f_d = nc.dram_tensor("fmat", [128, 128], bf16, kind="ExternalInput").ap()
    y_d = nc.dram_tensor("y", [B_SH, NO], f32, kind="ExternalOutput").ap()

    RG = [list(range(N_CORES))]

    with tile.TileContext(nc) as tc:
        with tc.tile_pool(name="persist", bufs=1) as pp, \
             tc.tile_pool(name="work", bufs=1) as wp, \
             tc.tile_pool(name="ps_s", bufs=2, space="PSUM") as ps_s, \
             tc.tile_pool(name="ps_q", bufs=3, space="PSUM") as ps_q, \
             tc.tile_pool(name="ps_f", bufs=1, space="PSUM") as ps_f, \
             tc.tile_pool(name="dram", bufs=1, space="DRAM") as dp:

            # ---------------- input loads ----------------
            xt_sb = pp.tile([128, NCH, B], bf16, name="xt_sb", tag="xt_sb")
            xik_sb = pp.tile([128, BC, JR], bf16, name="xik_sb", tag="xik_sb")
            wl_sb = pp.tile([128, NCH, NO], bf16, name="wl_sb", tag="wl_sb")
            f_sb = pp.tile([128, 128], bf16, name="f_sb", tag="f_sb")
            b_sb = pp.tile([128, NCH, N_NODE], f32, name="b_sb", tag="b_sb")

            xtf = xt_sb[:].rearrange("p c b -> p (c b)")
            wlf = wl_sb[:].rearrange("p c f -> p (c f)")
            nc.sync.dma_start(xtf[:, 0:5 * B], xt_d[:, 0:5 * B])
            nc.scalar.dma_start(wlf[:, 0:5 * NO], wl_d[:, 0:5 * NO])
            nc.sync.dma_start(wlf[:, 5 * NO:], wl_d[:, 5 * NO:])
            nc.scalar.dma_start(xtf[:, 5 * B:], xt_d[:, 5 * B:])

            wl4 = wl_sb[:].rearrange("p c (n o) -> p c n o", n=N_NODE)

            # ---------------- helpers ----------------
            def s_matmul(rhs3, ar_dsts, dt=f32):
                """ar_dsts[bc] (DRAM) = sum_c xt[:,c,bc].T @ rhs3[:,c,:]
                per batch-chunk: bc0's PSUM->SBUF copy + store DMA overlap
                bc1's matmuls (DMA cannot source PSUM directly)."""
                s_sb = wp.tile([128, BC, NO], dt, name="s_st" + str(dt),
                               tag="s_st" + str(dt))
                for bc_i in range(BC):
                    s_ps = ps_s.tile([128, NO], f32, name="s_ps", tag="s_ps")
                    for c in range(NCH):
                        nc.tensor.matmul(
                            s_ps[:],
                            xt_sb[:, c, bc_i * 128:(bc_i + 1) * 128],
                            rhs3[:, c, :],
                            start=(c == 0), stop=(c == NCH - 1))
                    if bc_i == 0:
                        nc.scalar.copy(s_sb[:, 0, :], s_ps[:])
                        nc.sync.dma_start(ar_dsts[0], s_sb[:, 0, :])
                    else:
                        nc.vector.tensor_copy(s_sb[:, 1, :], s_ps[:])
                        nc.scalar.dma_start(ar_dsts[1], s_sb[:, 1, :])

            def exchange_tiles(t, dt):
                ex_in = dp.tile([128, BC * NO], dt, name=f"ex_in{t}",
                                tag="ex_in")
                if EXCHANGE == "ag8":
                    ex_out = dp.tile([N_CORES * 128, BC * NO], dt,
                                     name=f"ex_out{t}", tag="ex_out",
                                     addr_space="Shared")
                else:
                    ex_out = dp.tile([128, BC * NO], dt, name=f"ex_out{t}",
                                     tag="ex_out", addr_space="Shared")
                return ex_in, ex_out

            def tree_reduce(ex_out):
                """fp8 AllGather output [8*128, 320] -> bf16 sum [128, 2, NO].
                Leafs split DVE(3)/GpSimd(1); fp8 reads are the cost."""
                agv = wp.tile([128, N_CORES, BC * NO], f8, name="agv",
                              tag="agv")
                ag3 = ex_out.rearrange("(r p) f -> p r f", p=128)
                nc.sync.dma_start(agv[:, 0:4, :], ag3[:, 0:4, :])
                nc.scalar.dma_start(agv[:, 4:8, :], ag3[:, 4:8, :])
                lf = wp.tile([128, 4, BC * NO], bf16, name="lf", tag="lf")
                for h in range(3):
                    nc.vector.tensor_add(lf[:, h, :], agv[:, 2 * h, :],
                                         agv[:, 2 * h + 1, :])
                nc.gpsimd.tensor_add(lf[:, 3, :], agv[:, 6, :], agv[:, 7, :])
                md = wp.tile([128, 2, BC * NO], bf16, name="md", tag="md")
                nc.vector.tensor_add(md[:], lf[:, 0:2, :], lf[:, 2:4, :])
                sfull = wp.tile([128, BC, NO], f32, name="sfull",
                                tag="sfull")
                nc.vector.tensor_add(
                    sfull[:].rearrange("p c f -> p (c f)"),
                    md[:, 0, :], md[:, 1, :])
                return sfull

            def rsqrt(msq, P, nch, tag, iters):
                """z ~ 1/sqrt(msq) via int bit-trick + Newton steps (DVE
                only -- avoids the Sqrt/Ln ACT table sets entirely)."""
                sh = [P, nch, N_NODE]
                zi = wp.tile(sh, i32, name="zi" + tag, tag="zi" + tag)
                nc.vector.tensor_scalar(
                    out=zi[:], in0=msq[:].bitcast(i32), scalar1=1, scalar2=-1,
                    op0=ALU.arith_shift_right, op1=ALU.bitwise_xor)
                nc.vector.tensor_scalar_add(zi[:], zi[:], RSQRT_MAGIC + 1)
                z = zi[:].bitcast(f32)
                t = wp.tile(sh, f32, name="nt" + tag, tag="nt" + tag)
                w = wp.tile(sh, f32, name="nw" + tag, tag="nw" + tag)
                for _ in range(iters):
                    nc.vector.tensor_mul(t[:], z, z)
                    nc.vector.tensor_mul(t[:], t[:], msq[:])
                    nc.vector.tensor_scalar(
                        out=w[:], in0=t[:], scalar1=-0.5, scalar2=1.5,
                        op0=ALU.mult, op1=ALU.add)
                    nc.vector.tensor_mul(z, z, w[:])
                return z

            def squash(s_ap, P, nch, tag, v_dtype, newton_iters, v_sb=None,
                       v_off=0, scale=None):
                """v = squash(s * scale) over o.  s_ap [P, nch, NO]."""
                s4 = s_ap.rearrange("p c (n o) -> p c n o", n=N_NODE)
                sq = wp.tile([P, nch, NO], f32, name="sq" + tag,
                             tag="sq" + tag)
                nc.vector.tensor_mul(sq[:], s_ap, s_ap)
                msq = wp.tile([P, nch, N_NODE], f32, name="msq" + tag,
                              tag="msq" + tag)
                nc.vector.reduce_sum(
                    msq[:], sq[:].rearrange("p c (n o) -> p c n o", n=N_NODE),
                    axis=AX.X)
                if scale is not None:
                    # s was pre-scale; msq *= scale^2 so fac comes out right,
                    # and the final v-mul absorbs scale via fac*scale.
                    nc.vector.tensor_scalar_mul(msq[:], msq[:],
                                                float(scale * scale))
                den = wp.tile([P, nch, N_NODE], f32, name="den" + tag,
                              tag="den" + tag)
                nc.vector.tensor_scalar_add(den[:], msq[:], 1.0)
                rden = wp.tile([P, nch, N_NODE], f32, name="rden" + tag,
                               tag="rden" + tag)
                nc.vector.reciprocal(rden[:], den[:])
                z = rsqrt(msq, P, nch, tag, newton_iters)
                mag = wp.tile([P, nch, N_NODE], f32, name="mag" + tag,
                              tag="mag" + tag)
                nc.vector.tensor_mul(mag[:], msq[:], z)   # sqrt(msq)
                fac = wp.tile([P, nch, N_NODE], f32, name="fac" + tag,
                              tag="fac" + tag)
                nc.vector.tensor_mul(fac[:], mag[:], rden[:])
                if scale is not None:
                    nc.vector.tensor_scalar_mul(fac[:], fac[:], float(scale))
                if v_sb is None:
                    v_sb = wp.tile([P, nch, NO], v_dtype, name="v" + tag,
                                   tag="v" + tag)
                    v4 = v_sb[:].rearrange("p c (n o) -> p c n o", n=N_NODE)
                else:
                    v4 = v_sb[:, v_off:v_off + nch, :].rearrange(
                        "p c (n o) -> p c n o", n=N_NODE)
                fb = fac[:].unsqueeze(3).broadcast_to((P, nch, N_NODE, O_SZ))
                nc.vector.tensor_mul(v4, s4, fb)
                return v_sb

            def squash_and_q(ex_out, scale=None):
                """Squash the exchanged s per batch-chunk, pipelined with the
                Q matmuls; then p = wl*Q (Q staged to bf16 SBUF by the scalar
                engine so the DVE multiply runs all-bf16)."""
                if EXCHANGE == "ag8":
                    sf = tree_reduce(ex_out)
                else:
                    sf = wp.tile([128, BC, NO], f32, name="sf", tag="sf")
                    for bc_i in range(BC):
                        eng = nc.sync if bc_i == 0 else nc.scalar
                        eng.dma_start(sf[:, bc_i, :],
                                      ex_out[:, bc_i * NO:(bc_i + 1) * NO])
                v_sb = wp.tile([128, BC, NO], bf16, name="v_m", tag="v_m")
                q_tiles = []
                for g in range(NCH // 3):
                    q_tiles.append(ps_q.tile([128, 3 * NO], f32, name="q_ps",
                                             tag="q_ps"))
                for bc_i in range(BC):
                    squash(sf[:, bc_i:bc_i + 1, :], 128, 1, "m",
                           bf16, NEWTON_ROUTE, v_sb=v_sb, v_off=bc_i,
                           scale=scale)
                    for g in range(NCH // 3):
                        for s_i in range(3):
                            mc = g * 3 + s_i
                            nc.tensor.matmul(
                                q_tiles[g][:, s_i * NO:(s_i + 1) * NO],
                                xik_sb[:, bc_i, mc * 128:(mc + 1) * 128],
                                v_sb[:, bc_i, :],
                                start=(bc_i == 0), stop=(bc_i == BC - 1))
                q_sb = wp.tile([128, NCH, NO], bf16, name="q_sb", tag="q_sb")
                p_sb = wp.tile([128, NCH, NO], bf16, name="p_sb", tag="p_sb")
                pr = wp.tile([128, NCH, N_NODE], f32, name="pr_sb",
                             tag="pr_sb")
                for g in range(NCH // 3):
                    gs = slice(g * 3, (g + 1) * 3)
                    nc.scalar.copy(
                        q_sb[:, gs, :].rearrange("p c f -> p (c f)"),
                        q_tiles[g][:])
                    nc.vector.tensor_mul(p_sb[:, gs, :], wl_sb[:, gs, :],
                                         q_sb[:, gs, :])
                    nc.vector.reduce_sum(
                        pr[:, gs, :],
                        p_sb[:, gs, :].rearrange(
                            "p c (n o) -> p c n o", n=N_NODE),
                        axis=AX.X)
                return v_sb, pr

            def b_update(pr, first):
                prb = wp.tile([128, NCH, N_NODE], bf16, name="prb", tag="prb")
                nc.vector.tensor_copy(prb[:], pr[:])
                uv_ps = ps_f.tile([128, NCH * N_NODE], f32, name="uv_ps",
                                  tag="uv_ps")
                nc.tensor.matmul(uv_ps[:], f_sb[:],
                                 prb[:].rearrange("p c n -> p (c n)"),
                                 start=True, stop=True)
                uv3 = uv_ps[:].rearrange("p (c n) -> p c n", n=N_NODE)
                if first:
                    # keep b state for the next update, but let the softmax
                    # read the PSUM uv directly (shorter critical path)
                    nc.scalar.copy(b_sb[:], uv3)
                    return uv3
                nc.vector.tensor_add(b_sb[:], b_sb[:], uv3)
                return b_sb[:]

            def softmax_mc(b_src):
                e_sb = wp.tile([128, NCH, N_NODE], f32, name="e_sb",
                               tag="e_sb")
                nc.scalar.activation(e_sb[:], b_src, AF.Exp)
                se = wp.tile([128, NCH], f32, name="se", tag="se")
                nc.vector.reduce_sum(se[:], e_sb[:], axis=AX.X)
                rse = wp.tile([128, NCH], f32, name="rse", tag="rse")
                nc.vector.reciprocal(rse[:], se[:])
                c_sb = wp.tile([128, NCH, N_NODE], bf16, name="c_sb",
                               tag="c_sb")
                nc.vector.tensor_mul(
                    c_sb[:], e_sb[:],
                    rse[:].unsqueeze(2).broadcast_to((128, NCH, N_NODE)))
                mc_sb = wp.tile([128, NCH, NO], bf16, name="mc_sb",
                                tag="mc_sb")
                cb = c_sb[:].unsqueeze(3).broadcast_to(
                    (128, NCH, N_NODE, O_SZ))
                mc4 = mc_sb[:].rearrange("p c (n o) -> p c n o", n=N_NODE)
                # split the W-sized multiply across DVE and the idle GpSimd
                nc.vector.tensor_mul(mc4[:, 0:8], wl4[:, 0:8], cb[:, 0:8])
                nc.gpsimd.tensor_mul(mc4[:, 8:NCH], wl4[:, 8:NCH],
                                     cb[:, 8:NCH])
                return mc_sb

            ex_dt = f8 if EXCHANGE == "ag8" else f32
            ex_kind = "AllGather" if EXCHANGE == "ag8" else "AllReduce"
            ex_op = ALU.bypass if EXCHANGE == "ag8" else ALU.add

            # -------- iteration 1 (c uniform = 0.1, folded into squash) ----
            ex_in, ex_out = exchange_tiles(0, ex_dt)
            s_matmul(wl_sb[:], [ex_in[:, 0:NO], ex_in[:, NO:2 * NO]],
                     dt=ex_dt)
            nc.gpsimd.collective_compute(
                ex_kind, ex_op, replica_groups=RG,
                ins=[ex_in.opt()], outs=[ex_out.opt()])
            # deferred inputs: queued behind the ex_in stores so they never
            # delay the first collective trigger
            nc.sync.dma_start(
                xik_sb[:].rearrange("p c j -> p (c j)"), xik_d[:])
            nc.scalar.dma_start(f_sb[:], f_d[:])
            v_sb, pr = squash_and_q(ex_out, scale=0.1)
            b_src = b_update(pr, first=True)

            # ---------------- iteration 2 ----------------
            mc_sb = softmax_mc(b_src)
            ex_in, ex_out = exchange_tiles(1, ex_dt)
            s_matmul(mc_sb[:], [ex_in[:, 0:NO], ex_in[:, NO:2 * NO]],
                     dt=ex_dt)
            nc.gpsimd.collective_compute(
                ex_kind, ex_op, replica_groups=RG,
                ins=[ex_in.opt()], outs=[ex_out.opt()])
            v_sb, pr = squash_and_q(ex_out)
            b_src = b_update(pr, first=False)

            # ---------------- iteration 3 (no b-update) ----------------
            mc_sb = softmax_mc(b_src)
            rs_in = dp.tile([B, NO], f32, name="rs_in", tag="rs_in")
            rs_out = dp.tile([B_SH, NO], f32, name="rs_out", tag="rs_out")
            rs2 = rs_in.rearrange("(c p) f -> p c f", p=128)
            s_matmul(mc_sb[:], [rs2[:, 0, :], rs2[:, 1, :]])
            nc.gpsimd.collective_compute(
                "ReduceScatter", ALU.add, replica_groups=RG,
                ins=[rs_in.opt()], outs=[rs_out.opt()])
            ssh = wp.tile([B_SH, 1, NO], f32, name="ssh", tag="ssh")
            nc.sync.dma_start(ssh[:, 0, :], rs_out[:])
            vsh = squash(ssh[:], B_SH, 1, "s", f32, 2)
            nc.sync.dma_start(y_d[:], vsh[:, 0, :])

    nc.compile()
    return nc


def _host_prep(x, W):
    """Per-core input dicts (partition-major layouts) + the F matrix."""
    import ml_dtypes

    bf = ml_dtypes.bfloat16
    x = np.ascontiguousarray(x, dtype=np.float32)
    W = np.ascontiguousarray(W, dtype=np.float32)
    F = (np.kron(np.eye(16, dtype=np.float32),
                 np.ones((8, 8), dtype=np.float32)) / np.float32(B)).astype(bf)
    in_maps = []
    for c in range(N_CORES):
        sl = slice(c * I_SH, (c + 1) * I_SH)
        x_sh = x[:, :, sl]                                   # [B, K, I_SH]
        # xt rows j=(i,k): [JR, B] -> partition-major [128, NCH, B]
        xt = np.ascontiguousarray(x_sh.transpose(2, 1, 0)).reshape(JR, B)
        xt_pm = np.ascontiguousarray(
            xt.reshape(NCH, 128, B).transpose(1, 0, 2)).reshape(128, NCH * B)
        # xik [B, JR] -> [128, BC, JR]
        xik = np.ascontiguousarray(
            x_sh.transpose(0, 2, 1)).reshape(B, JR)
        xik_pm = np.ascontiguousarray(
            xik.reshape(BC, 128, JR).transpose(1, 0, 2)).reshape(
                128, BC * JR)
        # wl rows j: [JR, NO] -> [128, NCH, NO]
        wlf = np.ascontiguousarray(
            (np.float32(0.03) * W[0, sl]).transpose(0, 3, 1, 2)
        ).reshape(JR, NO)
        wl_pm = np.ascontiguousarray(
            wlf.reshape(NCH, 128, NO).transpose(1, 0, 2)).reshape(
                128, NCH * NO)
        m = {"xt": xt_pm.astype(bf), "xik": xik_pm.astype(bf),
             "wl": wl_pm.astype(bf), "fmat": F}
        in_maps.append(m)
    return in_maps


def _run(in_maps, trace=False, all_cores=False):
    from concourse.bass_utils import run_bass_kernel_spmd

    if "nc" not in _CACHE:
        _CACHE["nc"] = _build_program()
    nc = _CACHE["nc"]
    kwargs = {}
    if all_cores:
        kwargs["trace_cores"] = list(range(N_CORES))
    res = run_bass_kernel_spmd(nc, in_maps, core_ids=list(range(N_CORES)),
                               trace=trace, **kwargs)
    return res


def kernel(x: np.ndarray, W: np.ndarray) -> np.ndarray:
    in_maps = _host_prep(x, W)
    res = _run(in_maps)
    v = np.concatenate([res.results[c]["y"] for c in range(N_CORES)], axis=0)
    return v.reshape(B, N_NODE, O_SZ, 1).astype(np.float32)


# revision 17
# speedup vs baseline: 1.3912x; 1.2198x over previous
"""CapsuleLayer (dynamic routing, 3 iterations) on 8 Trainium2 NeuronCores.

Decomposition (never materializes u_hat = [256,1152,10,16], 189MB):
  - Shard the 1152 input capsules (i) 8 ways: 144 per core.
  - Per-core row space j = (i_local, k), k = in_size = 8 -> 1152 rows
    = 9 chunks of 128 partitions.
  - s_j:  s[b,(n,o)] = sum_j xT[j,b] * (c[j,n] * Wl[j,(n,o)])   (PE matmul,
    contraction over j; Wl = 0.03*W in [(i,k),(n,o)] layout, c broadcast
    over k and o).  Partial over the i-shard -> summed across cores by a
    fp32 AllReduce straight out of PSUM (no SBUF copies, no on-chip tree).
  - b_ij update via a Gram matrix instead of u_hat:
       Q[j,(n,o)]  = sum_b x[b,j] * v[b,(n,o)]                  (PE matmul)
       pr[j,n]     = sum_o Wl[j,(n,o)] * Q[j,(n,o)]             (DVE)
       uv_rows     = F.T @ pr  per 128-chunk, F = kron(I16, ones8x8)/B
                     (sums over k within each i-group AND replicates the
                     result back to all k-rows, so b stays row-replicated)
  - Iteration 1 uses uniform c = 1/10 (softmax of zeros): s1 = 0.1*(xT.T@Wl).
  - Iteration 3 needs no b-update; the fp32 s3 goes through ReduceScatter
    (also straight out of PSUM) so each core squashes only its 32-row
    batch shard; the host just concatenates the 8 shards.

Latency plan (the kernel is serial-latency-bound, engines are <20% busy):
  - The 8-core rendezvous barrier releases when the LAST core triggers its
    first collective, so per-core time-to-first-trigger is on every core's
    critical path.  All inputs are host-transposed to partition-major so
    each loads with ONE plain 2D contiguous DMA (3D-strided dma_starts
    cost ~1.3-1.8us of descriptor generation each; 2D cost ~0.6us).
  - Routing matmuls use bf16 operands (fp32 PE matmuls are 4x slower).
  - squash per 128-batch-chunk is pipelined with the Q matmuls of the
    previous chunk; the W-sized mc = c*Wl multiply is split DVE/GpSimd.
  - sqrt is a bit-trick + Newton on the DVE so the ScalarE only ever needs
    the Exp activation table (Sqrt/Ln live in other table sets and would
    force ~2.7us ACT_TABLE_LOADs per iteration).  The routing squashes
    skip the Newton step entirely (~3.4% rsqrt error, which averages out
    across the 1152-capsule contraction); the output squash uses two.
"""
import sys

if "/opt/trn_rl_repo" not in sys.path:
    sys.path.insert(0, "/opt/trn_rl_repo")

import numpy as np

import os
N_CORES = int(os.environ.get("KERNEL_CORES", "8"))
B, IN_SIZE, I_TOT = 256, 8, 1152
N_NODE, O_SZ = 10, 16
NO = N_NODE * O_SZ          # 160
I_SH = I_TOT // N_CORES     # 144 capsules per core
JR = I_SH * IN_SIZE         # 1152 rows per core
NCH = JR // 128             # 9 contraction chunks
BC = B // 128               # 2 batch chunks
FCH = I_TOT * IN_SIZE // 128  # 72 full-row chunks (replicated iteration 1)
B_SH = B // N_CORES         # 32 batch rows per core after ReduceScatter

RSQRT_MAGIC = 0x5F3759DF
NEWTON_ROUTE = int(os.environ.get("KERNEL_NEWTON_ROUTE", "0"))
EXCHANGE = os.environ.get("KERNEL_EXCHANGE", "ag8")

_CACHE = {}


def _build_program():
    import concourse.bacc as bacc
    import concourse.tile as tile
    import concourse.mybir as mybir

    f32 = mybir.dt.float32
    bf16 = mybir.dt.bfloat16
    f8 = mybir.dt.float8e4
    i32 = mybir.dt.int32
    AF = mybir.ActivationFunctionType
    ALU = mybir.AluOpType
    AX = mybir.AxisListType

    nc = bacc.Bacc("TRN2", target_bir_lowering=False, debug=False,
                   enable_asserts=False, num_devices=N_CORES)

    # All inputs partition-major: one plain 2D contiguous DMA each.
    xf8_d = nc.dram_tensor("xf8", [128, FCH * B], f8,
                           kind="ExternalInput").ap()
    wf8_d = nc.dram_tensor("wf8", [128, FCH * NO], f8,
                           kind="ExternalInput").ap()
    xt_d = nc.dram_tensor("xt", [128, NCH * B], bf16,
                          kind="ExternalInput").ap()
    xik_d = nc.dram_tensor("xik", [128, BC * JR], bf16,
                           kind="ExternalInput").ap()
    wl_d = nc.dram_tensor("wl", [128, NCH * NO], bf16,
                          kind="ExternalInput").ap()
    f_d = nc.dram_tensor("fmat", [128, 128], bf16, kind="ExternalInput").ap()
    y_d = nc.dram_tensor("y", [B_SH, NO], f32, kind="ExternalOutput").ap()

    RG = [list(range(N_CORES))]

    with tile.TileContext(nc) as tc:
        with tc.tile_pool(name="persist", bufs=1) as pp, \
             tc.tile_pool(name="work", bufs=1) as wp, \
             tc.tile_pool(name="ps_s", bufs=2, space="PSUM") as ps_s, \
             tc.tile_pool(name="ps_q", bufs=3, space="PSUM") as ps_q, \
             tc.tile_pool(name="ps_f", bufs=1, space="PSUM") as ps_f, \
             tc.tile_pool(name="dram", bufs=1, space="DRAM") as dp:

            # ---------------- input loads ----------------
            # Replicated full-row tensors for the collective-free iteration 1
            # (fp8: halves the DMA, and s1 only steers routing).  The load +
            # s1 ride the fixed ~33us NEFF-init + CC-boot window, so the
            # first collective (AG of s2) triggers right at the floor.
            xf8_sb = pp.tile([128, FCH, B], f8, name="xf8_sb", tag="xf8_sb")
            wf8_sb = pp.tile([128, FCH, NO], f8, name="wf8_sb", tag="wf8_sb")
            xt_sb = pp.tile([128, NCH, B], bf16, name="xt_sb", tag="xt_sb")
            xik_sb = pp.tile([128, BC, JR], bf16, name="xik_sb", tag="xik_sb")
            wl_sb = pp.tile([128, NCH, NO], bf16, name="wl_sb", tag="wl_sb")
            f_sb = pp.tile([128, 128], bf16, name="f_sb", tag="f_sb")
            b_sb = pp.tile([128, NCH, N_NODE], f32, name="b_sb", tag="b_sb")

            xff = xf8_sb[:].rearrange("p c b -> p (c b)")
            wff = wf8_sb[:].rearrange("p c f -> p (c f)")
            H = FCH // 4
            # interleave xf8 quarters across sync/gpsimd so s1's chunk
            # groups land in consumption order; wf8 on scalar.
            nc.sync.dma_start(xff[:, 0:H * B], xf8_d[:, 0:H * B])
            nc.gpsimd.dma_start(xff[:, H * B:2 * H * B],
                                xf8_d[:, H * B:2 * H * B])
            nc.scalar.dma_start(wff[:, 0:2 * H * NO], wf8_d[:, 0:2 * H * NO])
            nc.sync.dma_start(xff[:, 2 * H * B:3 * H * B],
                              xf8_d[:, 2 * H * B:3 * H * B])
            nc.gpsimd.dma_start(xff[:, 3 * H * B:], xf8_d[:, 3 * H * B:])
            nc.scalar.dma_start(wff[:, 2 * H * NO:], wf8_d[:, 2 * H * NO:])
            # own-shard tensors (iterations 2-3): needed well after s1
            nc.sync.dma_start(
                xik_sb[:].rearrange("p c j -> p (c j)"), xik_d[:])
            nc.scalar.dma_start(f_sb[:], f_d[:])
            nc.scalar.dma_start(
                xt_sb[:].rearrange("p c b -> p (c b)"), xt_d[:])
            nc.scalar.dma_start(
                wl_sb[:].rearrange("p c f -> p (c f)"), wl_d[:])

            wl4 = wl_sb[:].rearrange("p c (n o) -> p c n o", n=N_NODE)

            # ---------------- helpers ----------------
            def s_matmul(rhs3, ar_dsts, dt=f32):
                """ar_dsts[bc] (DRAM) = sum_c xt[:,c,bc].T @ rhs3[:,c,:]
                per batch-chunk: bc0's PSUM->SBUF copy + store DMA overlap
                bc1's matmuls (DMA cannot source PSUM directly)."""
                s_sb = wp.tile([128, BC, NO], dt, name="s_st" + str(dt),
                               tag="s_st" + str(dt))
                for bc_i in range(BC):
                    s_ps = ps_s.tile([128, NO], f32, name="s_ps", tag="s_ps")
                    for c in range(NCH):
                        nc.tensor.matmul(
                            s_ps[:],
                            xt_sb[:, c, bc_i * 128:(bc_i + 1) * 128],
                            rhs3[:, c, :],
                            start=(c == 0), stop=(c == NCH - 1))
                    if bc_i == 0:
                        nc.scalar.copy(s_sb[:, 0, :], s_ps[:])
                        nc.sync.dma_start(ar_dsts[0], s_sb[:, 0, :])
                    else:
                        nc.vector.tensor_copy(s_sb[:, 1, :], s_ps[:])
                        nc.scalar.dma_start(ar_dsts[1], s_sb[:, 1, :])

            def exchange_tiles(t, dt):
                ex_in = dp.tile([128, BC * NO], dt, name=f"ex_in{t}",
                                tag="ex_in")
                if EXCHANGE == "ag8":
                    ex_out = dp.tile([N_CORES * 128, BC * NO], dt,
                                     name=f"ex_out{t}", tag="ex_out",
                                     addr_space="Shared")
                else:
                    ex_out = dp.tile([128, BC * NO], dt, name=f"ex_out{t}",
                                     tag="ex_out", addr_space="Shared")
                return ex_in, ex_out

            def tree_reduce(ex_out):
                """fp8 AllGather output [8*128, 320] -> bf16 sum [128, 2, NO].
                Leafs split DVE(3)/GpSimd(1); fp8 reads are the cost."""
                agv = wp.tile([128, N_CORES, BC * NO], f8, name="agv",
                              tag="agv")
                ag3 = ex_out.rearrange("(r p) f -> p r f", p=128)
                nc.sync.dma_start(agv[:, 0:4, :], ag3[:, 0:4, :])
                nc.scalar.dma_start(agv[:, 4:8, :], ag3[:, 4:8, :])
                lf = wp.tile([128, 4, BC * NO], bf16, name="lf", tag="lf")
                for h in range(3):
                    nc.vector.tensor_add(lf[:, h, :], agv[:, 2 * h, :],
                                         agv[:, 2 * h + 1, :])
                nc.gpsimd.tensor_add(lf[:, 3, :], agv[:, 6, :], agv[:, 7, :])
                md = wp.tile([128, 2, BC * NO], bf16, name="md", tag="md")
                nc.vector.tensor_add(md[:], lf[:, 0:2, :], lf[:, 2:4, :])
                sfull = wp.tile([128, BC, NO], f32, name="sfull",
                                tag="sfull")
                nc.vector.tensor_add(
                    sfull[:].rearrange("p c f -> p (c f)"),
                    md[:, 0, :], md[:, 1, :])
                return sfull

            def rsqrt(msq, P, nch, tag, iters):
                """z ~ 1/sqrt(msq) via int bit-trick + Newton steps (DVE
                only -- avoids the Sqrt/Ln ACT table sets entirely)."""
                sh = [P, nch, N_NODE]
                zi = wp.tile(sh, i32, name="zi" + tag, tag="zi" + tag)
                nc.vector.tensor_scalar(
                    out=zi[:], in0=msq[:].bitcast(i32), scalar1=1, scalar2=-1,
                    op0=ALU.arith_shift_right, op1=ALU.bitwise_xor)
                nc.vector.tensor_scalar_add(zi[:], zi[:], RSQRT_MAGIC + 1)
                z = zi[:].bitcast(f32)
                t = wp.tile(sh, f32, name="nt" + tag, tag="nt" + tag)
                w = wp.tile(sh, f32, name="nw" + tag, tag="nw" + tag)
                for _ in range(iters):
                    nc.vector.tensor_mul(t[:], z, z)
                    nc.vector.tensor_mul(t[:], t[:], msq[:])
                    nc.vector.tensor_scalar(
                        out=w[:], in0=t[:], scalar1=-0.5, scalar2=1.5,
                        op0=ALU.mult, op1=ALU.add)
                    nc.vector.tensor_mul(z, z, w[:])
                return z

            def squash(s_ap, P, nch, tag, v_dtype, newton_iters, v_sb=None,
                       v_off=0, scale=None):
                """v = squash(s * scale) over o.  s_ap [P, nch, NO]."""
                s4 = s_ap.rearrange("p c (n o) -> p c n o", n=N_NODE)
                sq = wp.tile([P, nch, NO], f32, name="sq" + tag,
                             tag="sq" + tag)
                nc.vector.tensor_mul(sq[:], s_ap, s_ap)
                msq = wp.tile([P, nch, N_NODE], f32, name="msq" + tag,
                              tag="msq" + tag)
                nc.vector.reduce_sum(
                    msq[:], sq[:].rearrange("p c (n o) -> p c n o", n=N_NODE),
                    axis=AX.X)
                if scale is not None:
                    # s was pre-scale; msq *= scale^2 so fac comes out right,
                    # and the final v-mul absorbs scale via fac*scale.
                    nc.vector.tensor_scalar_mul(msq[:], msq[:],
                                                float(scale * scale))
                den = wp.tile([P, nch, N_NODE], f32, name="den" + tag,
                              tag="den" + tag)
                nc.vector.tensor_scalar_add(den[:], msq[:], 1.0)
                rden = wp.tile([P, nch, N_NODE], f32, name="rden" + tag,
                               tag="rden" + tag)
                nc.vector.reciprocal(rden[:], den[:])
                z = rsqrt(msq, P, nch, tag, newton_iters)
                mag = wp.tile([P, nch, N_NODE], f32, name="mag" + tag,
                              tag="mag" + tag)
                nc.vector.tensor_mul(mag[:], msq[:], z)   # sqrt(msq)
                fac = wp.tile([P, nch, N_NODE], f32, name="fac" + tag,
                              tag="fac" + tag)
                nc.vector.tensor_mul(fac[:], mag[:], rden[:])
                if scale is not None:
                    nc.vector.tensor_scalar_mul(fac[:], fac[:], float(scale))
                if v_sb is None:
                    v_sb = wp.tile([P, nch, NO], v_dtype, name="v" + tag,
                                   tag="v" + tag)
                    v4 = v_sb[:].rearrange("p c (n o) -> p c n o", n=N_NODE)
                else:
                    v4 = v_sb[:, v_off:v_off + nch, :].rearrange(
                        "p c (n o) -> p c n o", n=N_NODE)
                fb = fac[:].unsqueeze(3).broadcast_to((P, nch, N_NODE, O_SZ))
                nc.vector.tensor_mul(v4, s4, fb)
                return v_sb

            def squash_and_q(ex_out, scale=None, direct=None):
                """Squash the exchanged s per batch-chunk, pipelined with the
                Q matmuls; then p = wl*Q (Q staged to bf16 SBUF by the scalar
                engine, split DVE/GpSimd)."""
                if direct is not None:
                    sf = direct
                elif EXCHANGE == "ag8":
                    sf = tree_reduce(ex_out)
                else:
                    sf = wp.tile([128, BC, NO], f32, name="sf", tag="sf")
                    for bc_i in range(BC):
                        eng = nc.sync if bc_i == 0 else nc.scalar
                        eng.dma_start(sf[:, bc_i, :],
                                      ex_out[:, bc_i * NO:(bc_i + 1) * NO])
                v_sb = wp.tile([128, BC, NO], bf16, name="v_m", tag="v_m")
                q_tiles = []
                for g in range(NCH // 3):
                    q_tiles.append(ps_q.tile([128, 3 * NO], f32, name="q_ps",
                                             tag="q_ps"))
                for bc_i in range(BC):
                    squash(sf[:, bc_i:bc_i + 1, :], 128, 1, "m",
                           bf16, NEWTON_ROUTE, v_sb=v_sb, v_off=bc_i,
                           scale=scale)
                    for g in range(NCH // 3):
                        for s_i in range(3):
                            mc = g * 3 + s_i
                            nc.tensor.matmul(
                                q_tiles[g][:, s_i * NO:(s_i + 1) * NO],
                                xik_sb[:, bc_i, mc * 128:(mc + 1) * 128],
                                v_sb[:, bc_i, :],
                                start=(bc_i == 0), stop=(bc_i == BC - 1))
                q_sb = wp.tile([128, NCH, NO], bf16, name="q_sb", tag="q_sb")
                p_sb = wp.tile([128, NCH, NO], bf16, name="p_sb", tag="p_sb")
                pr = wp.tile([128, NCH, N_NODE], f32, name="pr_sb",
                             tag="pr_sb")
                for g in range(NCH // 3):
                    gs = slice(g * 3, (g + 1) * 3)
                    g2 = slice(g * 3, g * 3 + 2)
                    nc.scalar.copy(
                        q_sb[:, gs, :].rearrange("p c f -> p (c f)"),
                        q_tiles[g][:])
                    nc.vector.tensor_mul(p_sb[:, g2, :], wl_sb[:, g2, :],
                                         q_sb[:, g2, :])
                    nc.gpsimd.tensor_mul(p_sb[:, g * 3 + 2, :],
                                         wl_sb[:, g * 3 + 2, :],
                                         q_sb[:, g * 3 + 2, :])
                    nc.vector.reduce_sum(
                        pr[:, gs, :],
                        p_sb[:, gs, :].rearrange(
                            "p c (n o) -> p c n o", n=N_NODE),
                        axis=AX.X)
                return v_sb, pr

            def b_update(pr, first):
                prb = wp.tile([128, NCH, N_NODE], bf16, name="prb", tag="prb")
                nc.vector.tensor_copy(prb[:], pr[:])
                uv_ps = ps_f.tile([128, NCH * N_NODE], f32, name="uv_ps",
                                  tag="uv_ps")
                nc.tensor.matmul(uv_ps[:], f_sb[:],
                                 prb[:].rearrange("p c n -> p (c n)"),
                                 start=True, stop=True)
                uv3 = uv_ps[:].rearrange("p (c n) -> p c n", n=N_NODE)
                if first:
                    # keep b state for the next update, but let the softmax
                    # read the PSUM uv directly (shorter critical path)
                    nc.scalar.copy(b_sb[:], uv3)
                    return uv3
                nc.vector.tensor_add(b_sb[:], b_sb[:], uv3)
                return b_sb[:]

            def softmax_mc(b_src):
                e_sb = wp.tile([128, NCH, N_NODE], f32, name="e_sb",
                               tag="e_sb")
                nc.scalar.activation(e_sb[:], b_src, AF.Exp)
                se = wp.tile([128, NCH], f32, name="se", tag="se")
                nc.vector.reduce_sum(se[:], e_sb[:], axis=AX.X)
                rse = wp.tile([128, NCH], f32, name="rse", tag="rse")
                nc.vector.reciprocal(rse[:], se[:])
                c_sb = wp.tile([128, NCH, N_NODE], bf16, name="c_sb",
                               tag="c_sb")
                nc.vector.tensor_mul(
                    c_sb[:], e_sb[:],
                    rse[:].unsqueeze(2).broadcast_to((128, NCH, N_NODE)))
                mc_sb = wp.tile([128, NCH, NO], bf16, name="mc_sb",
                                tag="mc_sb")
                cb = c_sb[:].unsqueeze(3).broadcast_to(
                    (128, NCH, N_NODE, O_SZ))
                mc4 = mc_sb[:].rearrange("p c (n o) -> p c n o", n=N_NODE)
                # split the W-sized multiply across DVE and the idle GpSimd
                nc.vector.tensor_mul(mc4[:, 0:8], wl4[:, 0:8], cb[:, 0:8])
                nc.gpsimd.tensor_mul(mc4[:, 8:NCH], wl4[:, 8:NCH],
                                     cb[:, 8:NCH])
                return mc_sb

            ex_dt = f8 if EXCHANGE == "ag8" else f32
            ex_kind = "AllGather" if EXCHANGE == "ag8" else "AllReduce"
            ex_op = ALU.bypass if EXCHANGE == "ag8" else ALU.add

            # -------- iteration 1: replicated full s1, no collective ------
            # s1_psum = sum_j xf8.T @ wf8 over ALL 9216 rows; wf8 = 8*Wl on
            # the host (dodges fp8 subnormals), so true s1 = (0.1/8)*s1_psum
            # -- the 0.0125 is folded into the squash.
            sf1 = wp.tile([128, BC, NO], f32, name="sf1", tag="sf1")
            for bc_i in range(BC):
                s_ps = ps_s.tile([128, NO], f32, name="s_ps", tag="s_ps")
                for c in range(FCH):
                    nc.tensor.matmul(
                        s_ps[:],
                        xf8_sb[:, c, bc_i * 128:(bc_i + 1) * 128],
                        wf8_sb[:, c, :],
                        start=(c == 0), stop=(c == FCH - 1))
                if bc_i == 0:
                    nc.scalar.copy(sf1[:, 0, :], s_ps[:])
                else:
                    nc.vector.tensor_copy(sf1[:, 1, :], s_ps[:])
            v_sb, pr = squash_and_q(None, scale=0.1 / 8.0, direct=sf1[:])
            b_src = b_update(pr, first=True)

            # ------------- iteration 2 (first collective) -----------------
            mc_sb = softmax_mc(b_src)
            ex_in, ex_out = exchange_tiles(0, ex_dt)
            s_matmul(mc_sb[:], [ex_in[:, 0:NO], ex_in[:, NO:2 * NO]],
                     dt=ex_dt)
            nc.gpsimd.collective_compute(
                ex_kind, ex_op, replica_groups=RG,
                ins=[ex_in.opt()], outs=[ex_out.opt()])
            v_sb, pr = squash_and_q(ex_out)
            b_src = b_update(pr, first=False)

            # ---------------- iteration 3 (no b-update) ----------------
            mc_sb = softmax_mc(b_src)
            rs_in = dp.tile([B, NO], f32, name="rs_in", tag="rs_in")
            rs_out = dp.tile([B_SH, NO], f32, name="rs_out", tag="rs_out")
            rs2 = rs_in.rearrange("(c p) f -> p c f", p=128)
            s_matmul(mc_sb[:], [rs2[:, 0, :], rs2[:, 1, :]])
            nc.gpsimd.collective_compute(
                "ReduceScatter", ALU.add, replica_groups=RG,
                ins=[rs_in.opt()], outs=[rs_out.opt()])
            ssh = wp.tile([B_SH, 1, NO], f32, name="ssh", tag="ssh")
            nc.sync.dma_start(ssh[:, 0, :], rs_out[:])
            vsh = squash(ssh[:], B_SH, 1, "s", f32, 1)
            nc.sync.dma_start(y_d[:], vsh[:, 0, :])

    nc.compile()
    return nc


def _host_prep(x, W):
    """Per-core input dicts (partition-major layouts) + the F matrix."""
    import ml_dtypes

    bf = ml_dtypes.bfloat16
    x = np.ascontiguousarray(x, dtype=np.float32)
    W = np.ascontiguousarray(W, dtype=np.float32)
    F = (np.kron(np.eye(16, dtype=np.float32),
                 np.ones((8, 8), dtype=np.float32)) / np.float32(B)).astype(bf)
    f8 = ml_dtypes.float8_e4m3
    # replicated full-row tensors, shard-major row order j=(core,i_loc,k)
    xt_full = np.ascontiguousarray(x.transpose(2, 1, 0)).reshape(
        I_TOT * IN_SIZE, B)
    xf8 = np.ascontiguousarray(
        xt_full.reshape(FCH, 128, B).transpose(1, 0, 2)).reshape(
            128, FCH * B).astype(f8)
    wl_full = np.ascontiguousarray(
        (np.float32(0.24) * W[0]).transpose(0, 3, 1, 2)).reshape(
            I_TOT * IN_SIZE, NO)
    wf8 = np.ascontiguousarray(
        wl_full.reshape(FCH, 128, NO).transpose(1, 0, 2)).reshape(
            128, FCH * NO).astype(f8)
    in_maps = []
    for c in range(N_CORES):
        sl = slice(c * I_SH, (c + 1) * I_SH)
        x_sh = x[:, :, sl]                                   # [B, K, I_SH]
        # xt rows j=(i,k): [JR, B] -> partition-major [128, NCH, B]
        xt = np.ascontiguousarray(x_sh.transpose(2, 1, 0)).reshape(JR, B)
        xt_pm = np.ascontiguousarray(
            xt.reshape(NCH, 128, B).transpose(1, 0, 2)).reshape(128, NCH * B)
        # xik [B, JR] -> [128, BC, JR]
        xik = np.ascontiguousarray(
            x_sh.transpose(0, 2, 1)).reshape(B, JR)
        xik_pm = np.ascontiguousarray(
            xik.reshape(BC, 128, JR).transpose(1, 0, 2)).reshape(
                128, BC * JR)
        # wl rows j: [JR, NO] -> [128, NCH, NO]
        wlf = np.ascontiguousarray(
            (np.float32(0.03) * W[0, sl]).transpose(0, 3, 1, 2)
        ).reshape(JR, NO)
        wl_pm = np.ascontiguousarray(
            wlf.reshape(NCH, 128, NO).transpose(1, 0, 2)).reshape(
                128, NCH * NO)
        m = {"xf8": xf8, "wf8": wf8, "xt": xt_pm.astype(bf),
             "xik": xik_pm.astype(bf), "wl": wl_pm.astype(bf), "fmat": F}
        in_maps.append(m)
    return in_maps


def _run(in_maps, trace=False, all_cores=False):
    from concourse.bass_utils import run_bass_kernel_spmd

    if "nc" not in _CACHE:
        _CACHE["nc"] = _build_program()
    nc = _CACHE["nc"]
    kwargs = {}
    if all_cores:
        kwargs["trace_cores"] = list(range(N_CORES))
    res = run_bass_kernel_spmd(nc, in_maps, core_ids=list(range(N_CORES)),
                               trace=trace, **kwargs)
    return res


def kernel(x: np.ndarray, W: np.ndarray) -> np.ndarray:
    in_maps = _host_prep(x, W)
    res = _run(in_maps)
    v = np.concatenate([res.results[c]["y"] for c in range(N_CORES)], axis=0)
    return v.reshape(B, N_NODE, O_SZ, 1).astype(np.float32)
